# revision 77
# baseline (speedup 1.0000x reference)
"""AdaFGL Bass kernel for 8 TRN2 NeuronCores.

Row-shards the N=6144 nodes across 8 cores (768 rows each). The dense
[N,N] relation matrix never touches HBM or SBUF: each core computes its
transposed column-block re^T[j, i_local] = t_full[j] . t_local[i] tile
by tile in PSUM (single-fp16 t, zero-padded to contract=128 so FWL
stays enabled) and immediately converts each tile to q = relu(re-m)
(fp16, scalar engine) and sigma = (q > 0) (fp16 0/1 step via is_gt on
the vector engine at 2x rate), which feed fp16 accumulation matmuls
(q@emb)^T and (sigma@emb)^T sharing one stationary emb chunk. The mean
m is available BEFORE the relation pass via the rank-1 identity
sum(re) = U.U - N with U = colsum(t), so the whole relation phase is
one fused, PE-dense loop. The block max is kept as a per-pair fp16
running max (tensor_max) so no single long reduce ever stalls the
sigma pipeline.

Signal algebra (a = prelu alpha, ip = 1/(mx-m), im = 1/m):
  pos_w = ip*z1 + a*z2 + (1-a)*emb
  neg_w = -a*ip*z1 + (1-a)*emb - z2
  z1 = qe + (m-1)*emb,  z2 = im*qe + se + emb - im*tw
with qe = (relu(re-m)@emb) incl raw diag, se = (step(re-m)@emb) incl
diag, tw = t(t^T emb) = re0@emb incl diag; all diagonal effects fold
into the emb coefficients (diag(re)=1). The max statistic excludes the
diagonal via 12 recomputed local-block tiles with a -BIG*I suppression
(rank-uniform code; the rank-dependent colmask input zeroes the raw
local-block slot). Because re is symmetric, block (k=7, r) equals
block (r, k=7) computed on core 7, so per-core max slots only cover
k<7 and the [1,1] max AllGather launches after k==6 - its whole round
trip overlaps the final fused chunk; its stats chain uses a gpsimd
partition_all_reduce (ext-isa library pre-warmed at startup).

Two pipelined AllGathers: AG-emb [emb fp16] fires right after the hete
MLP (its mesh hides under the softmax chain), AG-t [t^T fp16 | wr | u]
right after the pack; local-block max tiles + smooth MLP + ori MLP
fill the collective window. u is unpacked first as [8,64] across
partitions and reduced with two tiny PE matmuls so m gates the fused
start by <2us. Inputs are host-cast to fp16 (X, W matrices) so MLP
matmuls and transposes run at 1 cycle/row and input DMA bytes halve;
weight/X loads are batched 3D-AP DMAs spread across the sync/scalar/
gpsimd queues so compute starts ~8us after engine init.
"""

import sys, os
sys.path.insert(0, "/opt/trn_rl_repo")

import numpy as np
from contextlib import ExitStack

from concourse import bass, bacc, tile, mybir, bass_isa
from concourse.bass_utils import run_bass_kernel_spmd

F32 = mybir.dt.float32
F16 = mybir.dt.float16
AX = mybir.AxisListType
OP = mybir.AluOpType
AF = mybir.ActivationFunctionType

N = 6144
NCORES = 8
P = N // NCORES            # 768 rows per core
FEAT = 128
INSM = 512
HID = 256
OUT = 64
NT = P // 128              # 6 row tiles per core
NJ = N // 128              # 48 column chunks
# AG-emb payload: emb16 [128,384]f16 = 24576 f32 words
AGE = 24576
# AG-t payload (f32 words): tT16 | wr | u
OFF_W = 24576              # tT16 [64,768]f16
OFF_U = OFF_W + 4096       # wr [64,64]f32
AGW = OFF_U + 64           # u [1,64]f32
NSLOT = 19                 # 7 fused k-slots + 12 local suppressed
                           # (k=7 covered by peers via symmetry of re)
INV_N2 = 1.0 / float(N * N)
BIG = 1.0e6

_CACHE = {}


def _build():
    nc = bacc.Bacc("TRN2", target_bir_lowering=False, debug=False,
                   num_devices=NCORES)

    def din(name, shape, dt=F32):
        return nc.dram_tensor(name, list(shape), dt, kind="ExternalInput").ap()

    def dout(name, shape):
        return nc.dram_tensor(name, list(shape), F32, kind="ExternalOutput").ap()

    x_sm = din("x_sm16", (P, INSM), F16)
    x_ori = din("x_ori16", (P, FEAT), F16)
    Wh0 = din("Wh016", (INSM, HID), F16)
    Ws0 = din("Ws016", (INSM, HID), F16)
    Wl0 = din("Wl016", (FEAT, HID), F16)
    Wh1 = din("Wh116", (HID, OUT), F16)
    Ws1 = din("Ws116", (HID, OUT), F16)
    Wl1 = din("Wl116", (HID, OUT), F16)
    bh0 = din("b_hete0", (HID,)); bh1 = din("b_hete1", (OUT,))
    bs0 = din("b_smooth0", (HID,)); bs1 = din("b_smooth1", (OUT,))
    bl0 = din("b_local0", (HID,)); bl1 = din("b_local1", (OUT,))
    a_model = din("prelu_model", (1,))
    a_hete = din("prelu_hete", (1,))
    ident_d = din("ident", (128, 128))
    ident16_d = din("ident16", (128, 128), F16)
    ones_row_d = din("ones_row", (1, 128))
    ones_col_d = din("ones_col", (128, 1))
    colmask_d = din("colmask", (1, NSLOT))

    out_ori = dout("out_ori", (P, OUT))
    out_smooth = dout("out_smooth", (P, OUT))
    out_msg = dout("out_msg", (P, OUT))

    age_in = nc.dram_tensor("age_in", [1, AGE], F32).ap()
    age_out = nc.dram_tensor("age_out", [NCORES, AGE], F32,
                             addr_space="Shared").ap()
    ag_in = nc.dram_tensor("ag_in", [1, AGW], F32).ap()
    ag_out = nc.dram_tensor("ag_out", [NCORES, AGW], F32,
                            addr_space="Shared").ap()
    ag2_in = nc.dram_tensor("ag2_in", [1, 1], F32).ap()
    ag2_out = nc.dram_tensor("ag2_out", [NCORES, 1], F32,
                             addr_space="Shared").ap()

    with tile.TileContext(nc) as tc, ExitStack() as ctx:
        cp = ctx.enter_context(tc.tile_pool(name="const", bufs=1))

        # warm the gpsimd ext-isa library for partition_all_reduce (one-time
        # ~7.5us load) while input DMAs stream
        zz = cp.tile([128, 1], F32, tag="zz", name="zz")
        nc.vector.memset(zz[:, :], 0.0)
        zzo = cp.tile([128, 1], F32, tag="zzo", name="zzo")
        nc.gpsimd.partition_all_reduce(zzo[:, :], zz[:, :], channels=128,
                                       reduce_op=bass_isa.ReduceOp.max)

        # ---------- input DMAs: consts + x first, spread across queues ----
        ident16 = cp.tile([128, 128], F16, tag="ident16", name="ident16")
        nc.sync.dma_start(out=ident16[:, :], in_=ident16_d[:, :])
        ones_row = cp.tile([1, 128], F32, tag="ones_row", name="ones_row")
        nc.sync.dma_start(out=ones_row[:, :], in_=ones_row_d[:, :])
        ones_col = cp.tile([128, 1], F32, tag="ones_col", name="ones_col")
        nc.sync.dma_start(out=ones_col[:, :], in_=ones_col_d[:, :])
        asc = cp.tile([1, 2], F32, tag="asc", name="asc")
        nc.sync.dma_start(out=asc[:, 0:1], in_=a_model[0:1])
        nc.sync.dma_start(out=asc[:, 1:2], in_=a_hete[0:1])
        x16 = cp.tile([128, NT * INSM], F16, tag="x16", name="x16")
        x16v = x16[:, :].rearrange("p (i c) -> p i c", c=INSM)
        nc.sync.dma_start(
            out=x16v[:, 0:3, :],
            in_=x_sm[0:384, :].rearrange("(a p) c -> p a c", p=128))
        nc.scalar.dma_start(
            out=x16v[:, 3:6, :],
            in_=x_sm[384:768, :].rearrange("(a p) c -> p a c", p=128))

        # consts on gpsimd queue (must stay short: AG trigger lives here)
        ident = cp.tile([128, 128], F32, tag="ident", name="ident")
        nc.gpsimd.dma_start(out=ident[:, :], in_=ident_d[:, :])
        colmask = cp.tile([1, NSLOT], F32, tag="colmask", name="colmask")
        nc.gpsimd.dma_start(out=colmask[:, :], in_=colmask_d[:, :])

        # weights: batched DMAs on sync/scalar
        Wh0_t = cp.tile([128, 4 * HID], F16, tag="Wh0", name="Wh0")
        nc.sync.dma_start(
            out=Wh0_t[:, :].rearrange("p (a c) -> p a c", c=HID),
            in_=Wh0[:, :].rearrange("(a p) c -> p a c", p=128))
        Ws0_t = cp.tile([128, 4 * HID], F16, tag="Ws0", name="Ws0")
        nc.scalar.dma_start(
            out=Ws0_t[:, :].rearrange("p (a c) -> p a c", c=HID),
            in_=Ws0[:, :].rearrange("(a p) c -> p a c", p=128))
        Wh1_t = cp.tile([128, 2 * OUT], F16, tag="Wh1", name="Wh1")
        nc.sync.dma_start(
            out=Wh1_t[:, :].rearrange("p (a c) -> p a c", c=OUT),
            in_=Wh1[:, :].rearrange("(a p) c -> p a c", p=128))
        Ws1_t = cp.tile([128, 2 * OUT], F16, tag="Ws1", name="Ws1")
        nc.scalar.dma_start(
            out=Ws1_t[:, :].rearrange("p (a c) -> p a c", c=OUT),
            in_=Ws1[:, :].rearrange("(a p) c -> p a c", p=128))
        bh0_t = cp.tile([128, 2], F32, tag="bh0", name="bh0")
        nc.sync.dma_start(out=bh0_t[:, :].rearrange("p a -> p a"),
                          in_=bh0[:].rearrange("(a p) -> p a", p=128))
        bs0_t = cp.tile([128, 2], F32, tag="bs0", name="bs0")
        nc.scalar.dma_start(out=bs0_t[:, :].rearrange("p a -> p a"),
                            in_=bs0[:].rearrange("(a p) -> p a", p=128))
        bh1_r = cp.tile([1, OUT], F32, tag="bh1r", name="bh1r")
        nc.sync.dma_start(out=bh1_r[:, :], in_=bh1[:])
        bs1_r = cp.tile([1, OUT], F32, tag="bs1r", name="bs1r")
        nc.scalar.dma_start(out=bs1_r[:, :], in_=bs1[:])

        # ori branch loads (consumed late, in the AG2 window)
        x_o16 = cp.tile([128, NT * FEAT], F16, tag="xo16", name="xo16")
        nc.scalar.dma_start(
            out=x_o16[:, :].rearrange("p (a c) -> p a c", c=FEAT),
            in_=x_ori[:, :].rearrange("(a p) c -> p a c", p=128))
        Wl0_t = cp.tile([128, HID], F16, tag="Wl0", name="Wl0")
        nc.scalar.dma_start(out=Wl0_t[:, :], in_=Wl0[:, :])
        Wl1_t = cp.tile([128, 2 * OUT], F16, tag="Wl1", name="Wl1")
        nc.scalar.dma_start(
            out=Wl1_t[:, :].rearrange("p (a c) -> p a c", c=OUT),
            in_=Wl1[:, :].rearrange("(a p) c -> p a c", p=128))
        bl0_t = cp.tile([128, 2], F32, tag="bl0", name="bl0")
        nc.scalar.dma_start(out=bl0_t[:, :], in_=bl0[:].rearrange(
            "(a p) -> p a", p=128))
        bl1_r = cp.tile([1, OUT], F32, tag="bl1r", name="bl1r")
        nc.scalar.dma_start(out=bl1_r[:, :], in_=bl1[:])

        nbh0_t = cp.tile([128, 2], F32, tag="nbh0", name="nbh0")
        nc.vector.tensor_scalar(nbh0_t[:, :], bh0_t[:, :], -1.0, None, OP.mult)
        nbs0_t = cp.tile([128, 2], F32, tag="nbs0", name="nbs0")
        nc.vector.tensor_scalar(nbs0_t[:, :], bs0_t[:, :], -1.0, None, OP.mult)
        nbl0_t = cp.tile([128, 2], F32, tag="nbl0", name="nbl0")
        nc.vector.tensor_scalar(nbl0_t[:, :], bl0_t[:, :], -1.0, None, OP.mult)

        # ---------- broadcasts ----------
        aM = cp.tile([128, 1], F32, tag="aM", name="aM")
        aH = cp.tile([128, 1], F32, tag="aH", name="aH")
        naM = cp.tile([128, 1], F32, tag="naM", name="naM")
        naH = cp.tile([128, 1], F32, tag="naH", name="naH")
        bh1_b = cp.tile([128, OUT], F32, tag="bh1", name="bh1")
        bs1_b = cp.tile([128, OUT], F32, tag="bs1", name="bs1")
        bl1_b = cp.tile([128, OUT], F32, tag="bl1", name="bl1")
        with tc.tile_pool(name="ps_bc", bufs=2, space="PSUM") as ps0:
            for src, full, nfull in ((asc[:, 0:1], aM, naM),
                                     (asc[:, 1:2], aH, naH)):
                ps = ps0.tile([128, 1], F32, tag="ps_bc", name="ps_bc")
                nc.tensor.matmul(ps[:, :], ones_row[:, :], src,
                                 start=True, stop=True)
                nc.scalar.copy(full[:, :], ps[:, :])
                nc.vector.tensor_scalar(nfull[:, :], full[:, :], -1.0, None,
                                        OP.mult)
            for row, full in ((bh1_r, bh1_b), (bs1_r, bs1_b),
                              (bl1_r, bl1_b)):
                ps = ps0.tile([128, OUT], F32, tag="ps_b1", name="ps_b1")
                nc.tensor.matmul(ps[:, :], ones_row[:, :], row[:, :],
                                 start=True, stop=True)
                nc.scalar.copy(full[:, :], ps[:, :])

        # ---------- transpose x_sm (fp16, 1 cyc/row) ----------
        XT = [cp.tile([128, P], F16, tag=f"xt{f}", name=f"xt{f}")
              for f in range(4)]
        with tc.tile_pool(name="ps_tr", bufs=4, space="PSUM") as pst:
            for i in range(NT):
                for f in range(4):
                    pt = pst.tile([128, 128], F16, tag="tr", name="tr")
                    nc.tensor.transpose(
                        pt[:, :], x16v[:, i, 128 * f:128 * (f + 1)],
                        ident16[:, :])
                    if f % 2 == 0:
                        nc.scalar.copy(XT[f][:, 128 * i:128 * (i + 1)],
                                       pt[:, :])
                    else:
                        nc.vector.tensor_copy(XT[f][:, 128 * i:128 * (i + 1)],
                                              pt[:, :])

        def esl(i):
            return slice(OUT * i, OUT * (i + 1))

        # ---------- MLP helper (fp16 l1+l2 weights/inputs) ----------
        def mlp(XT_tiles, w0, b0t, nb0t, w1, b1_b, na_b, out_wide, pfx):
            with tc.tile_pool(name=pfx + "_h", bufs=1) as hp, \
                 tc.tile_pool(name=pfx + "_r", bufs=2) as rp, \
                 tc.tile_pool(name=pfx + "_ps1", bufs=2, space="PSUM") as ps1, \
                 tc.tile_pool(name=pfx + "_ps2", bufs=2, space="PSUM") as ps2:
                h_tiles = []
                for hs in range(2):
                    h = hp.tile([128, P], F16, tag=f"h{hs}",
                                name=f"{pfx}h{hs}")
                    for ns2 in range(2):
                        sl = slice(384 * ns2, 384 * (ns2 + 1))
                        pp = ps1.tile([128, 384], F32, tag="l1", name="l1")
                        nf = len(XT_tiles)
                        for fc in range(nf):
                            nc.tensor.matmul(pp[:, :], w0(fc, hs),
                                             XT_tiles[fc][:, sl],
                                             start=(fc == 0),
                                             stop=(fc == nf - 1))
                        r1 = rp.tile([128, 384], F32, tag="r1", name="r1")
                        nc.scalar.activation(r1[:, :], pp[:, :], AF.Relu,
                                             bias=b0t[:, hs:hs + 1],
                                             scale=1.0)
                        r2 = rp.tile([128, 384], F32, tag="r2", name="r2")
                        nc.scalar.activation(r2[:, :], pp[:, :], AF.Relu,
                                             bias=nb0t[:, hs:hs + 1],
                                             scale=-1.0)
                        nc.vector.scalar_tensor_tensor(
                            h[:, sl], r2[:, :], na_b[:, :], r1[:, :],
                            op0=OP.mult, op1=OP.add)
                    h_tiles.append(h)
                for i in range(NT):
                    pp = ps2.tile([128, OUT], F32, tag="l2", name="l2")
                    for hs in range(2):
                        nc.tensor.matmul(pp[:, :],
                                         h_tiles[hs][:, 128 * i:128 * (i + 1)],
                                         w1(hs), start=(hs == 0),
                                         stop=(hs == 1))
                    nc.vector.scalar_tensor_tensor(
                        out_wide[:, esl(i)], pp[:, :], 1.0, b1_b[:, :],
                        op0=OP.mult, op1=OP.add)

        def w0h(fc, hs):
            return Wh0_t[:, HID * fc + 128 * hs:HID * fc + 128 * (hs + 1)]

        def w0s(fc, hs):
            return Ws0_t[:, HID * fc + 128 * hs:HID * fc + 128 * (hs + 1)]

        def w1h(hs):
            return Wh1_t[:, OUT * hs:OUT * (hs + 1)]

        def w1s(hs):
            return Ws1_t[:, OUT * hs:OUT * (hs + 1)]

        # persistent phase-1 outputs
        emb_loc = cp.tile([128, NT * OUT], F32, tag="emb_loc", name="emb_loc")
        emb16_loc = cp.tile([128, NT * OUT], F16, tag="e16l", name="e16l")
        tT16 = cp.tile([128, P], F16, tag="tT16", name="tT16")
        tw = cp.tile([128, NT * OUT], F32, tag="tw", name="tw")
        maxs = cp.tile([128, NSLOT], F32, tag="maxs", name="maxs")
        nc.vector.memset(tT16[64:128, :], 0.0)

        # hete MLP first (feeds the collective)
        mlp(XT, w0h, bh0_t, nbh0_t, w1h, bh1_b, naH, emb_loc, "hete")
        nc.vector.tensor_copy(emb16_loc[:, :], emb_loc[:, :])

        # AG-emb goes out as soon as the hete MLP is done
        nc.sync.dma_start(out=age_in[0:1, 0:AGE],
                          in_=emb16_loc[:, :].bitcast(F32))
        nc.gpsimd.collective_compute(
            "AllGather", OP.bypass,
            ins=[age_in[:, :]],
            outs=[age_out[:, :]],
            replica_groups=[list(range(NCORES))])

        # ---------- softmax -> t = e/||e|| (fp16), wr, u ----------
        wr_sb = cp.tile([64, 64], F32, tag="wr_sb", name="wr_sb")
        u_sb = cp.tile([1, 64], F32, tag="u_sb", name="u_sb")
        with tc.tile_pool(name="smax", bufs=1) as sp, \
             tc.tile_pool(name="ps_wr", bufs=1, space="PSUM") as pswr, \
             tc.tile_pool(name="ps_tr2", bufs=2, space="PSUM") as pst2:
            rmx = sp.tile([128, NT], F32, tag="rmx", name="rmx")
            nc.vector.tensor_reduce(
                rmx[:, :],
                emb_loc[:, :].rearrange("p (g o) -> p g o", o=OUT),
                axis=AX.X, op=OP.max, negate=True)
            ex_w = sp.tile([128, NT * OUT], F32, tag="ex_w", name="ex_w")
            for i in range(NT):
                nc.scalar.activation(ex_w[:, esl(i)], emb_loc[:, esl(i)],
                                     AF.Exp, bias=rmx[:, i:i + 1], scale=1.0)
            sq_w = sp.tile([128, NT * OUT], F32, tag="sq_w", name="sq_w")
            nc.vector.tensor_mul(sq_w[:, :], ex_w[:, :], ex_w[:, :])
            dsum = sp.tile([128, NT], F32, tag="dsum", name="dsum")
            nc.vector.tensor_reduce(
                dsum[:, :],
                sq_w[:, :].rearrange("p (g o) -> p g o", o=OUT),
                axis=AX.X, op=OP.add)
            rd = sp.tile([128, NT], F32, tag="rd", name="rd")
            nc.vector.reciprocal(rd[:, :], dsum[:, :])
            isd = sp.tile([128, NT], F32, tag="isd", name="isd")
            nc.scalar.activation(isd[:, :], rd[:, :], AF.Sqrt)
            t16 = sp.tile([128, NT * OUT], F16, tag="t16", name="t16")
            for i in range(NT):
                nc.vector.tensor_scalar(t16[:, esl(i)], ex_w[:, esl(i)],
                                        isd[:, i:i + 1], None, OP.mult)
            ps_wr = pswr.tile([64, 64], F32, tag="wr", name="pswr")
            for i in range(NT):
                nc.tensor.matmul(ps_wr[:, :], t16[:, esl(i)],
                                 emb16_loc[:, esl(i)],
                                 start=(i == 0), stop=(i == NT - 1))
                pt = pst2.tile([64, 128], F16, tag="ttr", name="ttr")
                nc.tensor.transpose(pt[:, :], t16[:, esl(i)], ident16[:, :])
                nc.scalar.copy(tT16[0:64, 128 * i:128 * (i + 1)], pt[:, :])
            nc.scalar.copy(wr_sb[:, :], ps_wr[:, :])
            # u = colsum(t): rowsum of tT16, transposed to a row
            uT = sp.tile([64, 1], F32, tag="uT", name="uT")
            nc.vector.tensor_reduce(uT[:, :], tT16[0:64, :], axis=AX.X,
                                    op=OP.add)
            pu = pst2.tile([1, 64], F32, tag="put", name="put")
            nc.tensor.transpose(pu[:, :], uT[:, :], ident[0:64, 0:64])
            nc.scalar.copy(u_sb[:, :], pu[:, :])

        # ---------- pack + AG-t ----------
        nc.sync.dma_start(out=ag_in[0:1, 0:OFF_W],
                          in_=tT16[0:64, :].bitcast(F32))
        nc.sync.dma_start(out=ag_in[0:1, OFF_W:OFF_U], in_=wr_sb[:, :])
        nc.scalar.dma_start(out=ag_in[0:1, OFF_U:AGW], in_=u_sb[:, :])
        nc.gpsimd.collective_compute(
            "AllGather", OP.bypass,
            ins=[ag_in[:, :]],
            outs=[ag_out[:, :]],
            replica_groups=[list(range(NCORES))])

        # ---------- AG bubble: local-block max tiles + smooth MLP ----------
        negbigI = cp.tile([128, 128], F32, tag="negbigI", name="negbigI")
        nc.vector.tensor_scalar(negbigI[:, :], ident[:, :], -BIG, None,
                                OP.mult)
        with tc.tile_pool(name="ps_rex", bufs=2, space="PSUM") as psre:
            for s in range(NT):
                for h in range(2):
                    sl = slice(384 * h, 384 * (h + 1))
                    pp = psre.tile([128, 384], F32, tag="rex", name="rex")
                    nc.tensor.matmul(pp[:, :],
                                     tT16[:, 128 * s:128 * (s + 1)],
                                     tT16[:, sl], start=True, stop=True)
                    if (s // 3) == h:
                        off = 128 * s - 384 * h
                        nc.vector.scalar_tensor_tensor(
                            pp[:, off:off + 128], ident[:, :], -BIG,
                            pp[:, off:off + 128], op0=OP.mult, op1=OP.add)
                    slot = 7 + 2 * s + h
                    nc.vector.tensor_reduce(maxs[:, slot:slot + 1], pp[:, :],
                                            axis=AX.X, op=OP.max)
        with tc.tile_pool(name="mlpout", bufs=1) as mo:
            sm_out = mo.tile([128, NT * OUT], F32, tag="smo", name="smo")
            mlp(XT, w0s, bs0_t, nbs0_t, w1s, bs1_b, naM, sm_out, "smooth")
            nc.sync.dma_start(
                out=out_smooth[:, :].rearrange("(a p) c -> p a c", p=128),
                in_=sm_out[:, :].rearrange("p (a c) -> p a c", c=OUT))

        # ori MLP also fills the AG window
        XTo = cp.tile([128, P], F16, tag="xto", name="xto")
        with tc.tile_pool(name="ps_or", bufs=2, space="PSUM") as pso:
            for i in range(NT):
                pt = pso.tile([128, 128], F16, tag="otr", name="otr")
                nc.tensor.transpose(
                    pt[:, :],
                    x_o16[:, :].rearrange("p (a c) -> p a c",
                                          c=FEAT)[:, i, :],
                    ident16[:, :])
                nc.scalar.copy(XTo[:, 128 * i:128 * (i + 1)], pt[:, :])

        def w0l(fc, hs):
            return Wl0_t[:, 128 * hs:128 * (hs + 1)]

        def w1l(hs):
            return Wl1_t[:, OUT * hs:OUT * (hs + 1)]

        with tc.tile_pool(name="mlpout2", bufs=1) as mo2:
            or_out = mo2.tile([128, NT * OUT], F32, tag="oro", name="oro")
            mlp([XTo], w0l, bl0_t, nbl0_t, w1l, bl1_b, naM, or_out, "ori")
            nc.sync.dma_start(
                out=out_ori[:, :].rearrange("(a p) c -> p a c", p=128),
                in_=or_out[:, :].rearrange("p (a c) -> p a c", c=OUT))

        # ---------- unpack AG-emb + AG-t ----------
        tf16 = [cp.tile([128, P], F16, tag=f"tf{k}", name=f"tf{k}")
                for k in range(NCORES)]
        emb16 = [cp.tile([128, NT * OUT], F16, tag=f"e16_{k}",
                         name=f"e16_{k}") for k in range(NCORES)]
        w16 = cp.tile([64, 64], F16, tag="w16", name="w16")
        mb = cp.tile([128, 1], F32, tag="mb", name="mb")
        nmb = cp.tile([128, 1], F32, tag="nmb", name="nmb")
        # u gather FIRST: it unlocks the m chain / fused start.
        # uall8 [8,64] across partitions; U via two tiny PE matmuls.
        with tc.tile_pool(name="unpack", bufs=1) as up, \
             tc.tile_pool(name="ps_m", bufs=2, space="PSUM") as psm:
            uall8 = up.tile([8, 64], F32, tag="uall8", name="uall8")
            nc.sync.dma_start(out=uall8[:, :], in_=ag_out[:, OFF_U:AGW])
            pU = psm.tile([64, 1], F32, tag="pU", name="pU")
            nc.tensor.matmul(pU[:, :], uall8[:, :], ones_col[0:8, :],
                             start=True, stop=True)
            UT = up.tile([64, 1], F32, tag="UT", name="UT")
            nc.scalar.copy(UT[:, :], pU[:, :])
            puu = psm.tile([1, 1], F32, tag="puu", name="puu")
            nc.tensor.matmul(puu[:, :], UT[:, :], UT[:, :],
                             start=True, stop=True)
            m01 = up.tile([1, 1], F32, tag="m01", name="m01")
            nc.vector.tensor_scalar(m01[:, :], puu[:, :], -float(N), INV_N2,
                                    OP.add, OP.mult)
            pb = psm.tile([128, 1], F32, tag="mbc", name="mbc")
            nc.tensor.matmul(pb[:, :], ones_row[:, :], m01[:, :],
                             start=True, stop=True)
            nc.scalar.copy(mb[:, :], pb[:, :])
            nc.vector.tensor_scalar(nmb[:, :], mb[:, :], -1.0, None, OP.mult)
            # big per-core unpacks (k-ascending: the fused loop chases them)
            for k in range(NCORES):
                nc.vector.memset(tf16[k][64:128, :], 0.0)
                eng = nc.sync if k % 2 == 0 else nc.scalar
                eng2 = nc.scalar if k % 2 == 0 else nc.sync
                eng.dma_start(out=tf16[k][0:64, :].bitcast(F32),
                              in_=ag_out[k:k + 1, 0:OFF_W])
                eng2.dma_start(out=emb16[k][:, :].bitcast(F32),
                               in_=age_out[k:k + 1, 0:AGE])
            # wr gather (needed only in the epilogue)
            wrall = up.tile([64, 8 * 64], F32, tag="wrall", name="wrall")
            nc.scalar.dma_start(
                out=wrall[:, :].rearrange("p (k c) -> p k c", k=NCORES),
                in_=ag_out[:, OFF_W:OFF_U].rearrange("k (p c) -> p k c",
                                                     p=64))
            w_sb = up.tile([64, 64], F32, tag="w_sb", name="w_sb")
            nc.vector.tensor_reduce(
                w_sb[:, :],
                wrall[:, :].rearrange("p (k c) -> p c k", k=NCORES),
                axis=AX.X, op=OP.add)
            nc.vector.tensor_copy(w16[:, :], w_sb[:, :])

        # epilogue scalars that only need m / aH
        nimb = cp.tile([128, 1], F32, tag="nimb", name="nimb")  # -1/m
        imb = cp.tile([128, 1], F32, tag="imb", name="imb")     # 1/m
        omaH = cp.tile([128, 1], F32, tag="omaH", name="omaH")  # 1-aH
        n1m = cp.tile([128, 1], F32, tag="n1m", name="n1m")     # m-1
        nc.vector.reciprocal(imb[:, :], mb[:, :])
        nc.vector.tensor_scalar(nimb[:, :], imb[:, :], -1.0, None, OP.mult)
        nc.vector.tensor_scalar(omaH[:, :], aH[:, :], -1.0, 1.0, OP.mult,
                                OP.add)
        nc.vector.tensor_scalar(n1m[:, :], mb[:, :], 1.0, -1.0, OP.mult,
                                OP.add)

        # ---------- fused relation + propagation pass ----------
        qeT = cp.tile([64, P], F32, tag="qeT", name="qeT")
        seT = cp.tile([64, P], F32, tag="seT", name="seT")
        with tc.tile_pool(name="ps_re", bufs=4, space="PSUM") as psre, \
             tc.tile_pool(name="ps_acc", bufs=1, space="PSUM") as pacc, \
             tc.tile_pool(name="qc", bufs=3) as qcp, \
             tc.tile_pool(name="sg", bufs=3) as sgp, \
             tc.tile_pool(name="rmp", bufs=2) as rmp:
            qe_ps = [pacc.tile([64, 384], F32, tag=f"qe{h}", name=f"qe{h}")
                     for h in range(2)]
            se_ps = [pacc.tile([64, 384], F32, tag=f"se{h}", name=f"se{h}")
                     for h in range(2)]
            for k in range(NCORES):
                rm = rmp.tile([128, P], F16, tag="rm", name="rm")
                for sub in range(NT):
                    j = NT * k + sub
                    q2 = qcp.tile([128, P], F16, tag="q2", name="q2")
                    s2 = sgp.tile([128, P], F16, tag="s2", name="s2")
                    pps = []
                    for h in range(2):
                        pp = psre.tile([128, 384], F32, tag="rem",
                                       name="rem")
                        nc.tensor.matmul(
                            pp[:, :],
                            tf16[k][:, 128 * sub:128 * (sub + 1)],
                            tT16[:, 384 * h:384 * (h + 1)],
                            start=True, stop=True)
                        pps.append(pp)
                    for h in range(2):
                        qsl = slice(384 * h, 384 * (h + 1))
                        nc.scalar.activation(q2[:, qsl], pps[h][:, :],
                                             AF.Relu, bias=nmb[:, :],
                                             scale=1.0)
                    nc.vector.tensor_scalar(s2[:, :], q2[:, :], 0.0,
                                            None, OP.is_gt)
                    if k < 7:
                        if sub == 0:
                            nc.vector.tensor_scalar(rm[:, :], q2[:, :],
                                                    0.0, None, OP.max)
                        else:
                            nc.vector.tensor_max(rm[:, :], rm[:, :],
                                                 q2[:, :])
                    for h in range(2):
                        qsl = slice(384 * h, 384 * (h + 1))
                        nc.tensor.matmul(qe_ps[h][:, :],
                                         emb16[k][:, esl(sub)], q2[:, qsl],
                                         start=(j == 0), stop=(j == NJ - 1),
                                         skip_group_check=True)
                        nc.tensor.matmul(se_ps[h][:, :],
                                         emb16[k][:, esl(sub)], s2[:, qsl],
                                         start=(j == 0), stop=(j == NJ - 1),
                                         skip_group_check=True)
                if k < 7:
                    nc.vector.tensor_reduce(maxs[:, k:k + 1], rm[:, :],
                                            axis=AX.X, op=OP.max)
                if k == 6:
                    # k=7 slot is covered by peers (re is symmetric):
                    # stats + AG2 launch overlap the last fused chunk
                    nc.vector.tensor_scalar(maxs[:, 7:NSLOT],
                                            maxs[:, 7:NSLOT], mb[:, :],
                                            0.0, OP.subtract, OP.max)
                    mall = cp.tile([128, NSLOT], F32, tag="mall",
                                   name="mall")
                    nc.gpsimd.partition_all_reduce(
                        mall[:, :], maxs[:, :], channels=128,
                        reduce_op=bass_isa.ReduceOp.max)
                    mrow = cp.tile([1, NSLOT], F32, tag="mrow", name="mrow")
                    nc.vector.tensor_mul(mrow[:, :], mall[0:1, :],
                                         colmask[:, :])
                    mx01 = cp.tile([1, 1], F32, tag="mx01", name="mx01")
                    nc.vector.tensor_reduce(mx01[:, :], mrow[:, :],
                                            axis=AX.X, op=OP.max)
                    nc.sync.dma_start(out=ag2_in[:, :], in_=mx01[:, :])
                    nc.gpsimd.collective_compute(
                        "AllGather", OP.bypass,
                        ins=[ag2_in[:, :]],
                        outs=[ag2_out[:, :]],
                        replica_groups=[list(range(NCORES))])
            for h in range(2):
                sl = slice(384 * h, 384 * (h + 1))
                nc.scalar.copy(qeT[:, sl], qe_ps[h][:, :])
                nc.scalar.copy(seT[:, sl], se_ps[h][:, :])

        # ---------- epilogue prep (independent of qmax) ----------
        with tc.tile_pool(name="epi", bufs=1) as ep, \
             tc.tile_pool(name="ps_epi", bufs=2, space="PSUM") as pse:
            qe_nm = ep.tile([128, NT * OUT], F32, tag="qe_nm", name="qe_nm")
            se_nm = ep.tile([128, NT * OUT], F32, tag="se_nm", name="se_nm")
            for i in range(NT):
                pp = pse.tile([128, OUT], F32, tag="twp", name="twp")
                nc.tensor.matmul(pp[:, :], tT16[0:64, 128 * i:128 * (i + 1)],
                                 w16[:, :], start=True, stop=True)
                nc.scalar.copy(tw[:, esl(i)], pp[:, :])
            for i in range(NT):
                csl = slice(128 * i, 128 * (i + 1))
                pq = pse.tile([128, 64], F32, tag="tq", name="tq")
                nc.tensor.transpose(pq[:, :], qeT[:, csl], ident[0:64, 0:64])
                nc.scalar.copy(qe_nm[:, esl(i)], pq[:, :])
                pc = pse.tile([128, 64], F32, tag="tc", name="tc")
                nc.tensor.transpose(pc[:, :], seT[:, csl], ident[0:64, 0:64])
                nc.scalar.copy(se_nm[:, esl(i)], pc[:, :])

            z1 = ep.tile([128, NT * OUT], F32, tag="z1", name="z1")
            nc.vector.scalar_tensor_tensor(z1[:, :], emb_loc[:, :], n1m[:, :],
                                           qe_nm[:, :], op0=OP.mult,
                                           op1=OP.add)
            z2 = ep.tile([128, NT * OUT], F32, tag="z2", name="z2")
            nc.vector.tensor_scalar(z2[:, :], qe_nm[:, :], imb[:, :], None,
                                    OP.mult)
            nc.vector.tensor_add(z2[:, :], z2[:, :], se_nm[:, :])
            nc.vector.scalar_tensor_tensor(z2[:, :], tw[:, :], nimb[:, :],
                                           z2[:, :], op0=OP.mult, op1=OP.add)
            nc.vector.tensor_add(z2[:, :], z2[:, :], emb_loc[:, :])
            t2 = ep.tile([128, NT * OUT], F32, tag="t2", name="t2")
            nc.vector.tensor_scalar(t2[:, :], emb_loc[:, :], omaH[:, :],
                                    None, OP.mult)
            cpos = ep.tile([128, NT * OUT], F32, tag="cpos", name="cpos")
            nc.vector.scalar_tensor_tensor(cpos[:, :], z2[:, :], aH[:, :],
                                           t2[:, :], op0=OP.mult, op1=OP.add)
            cneg = ep.tile([128, NT * OUT], F32, tag="cneg", name="cneg")
            nc.vector.tensor_sub(cneg[:, :], t2[:, :], z2[:, :])
            ehalf = ep.tile([128, NT * OUT], F32, tag="ehalf", name="ehalf")
            nc.vector.tensor_scalar(ehalf[:, :], emb_loc[:, :], 0.5, None,
                                    OP.mult)

            # ---------- qmax -> ip ----------
            ipb = cp.tile([128, 1], F32, tag="ipb", name="ipb")
            naip = cp.tile([128, 1], F32, tag="naip", name="naip")
            with tc.tile_pool(name="glob", bufs=1) as gp, \
                 tc.tile_pool(name="ps_gl", bufs=1, space="PSUM") as psg:
                m8 = gp.tile([1, 8], F32, tag="m8", name="m8")
                nc.sync.dma_start(out=m8[:, :], in_=ag2_out[:, 0:1])
                mxs = gp.tile([1, 1], F32, tag="mxs", name="mxs")
                nc.vector.tensor_reduce(mxs[:, :], m8[:, :], axis=AX.X,
                                        op=OP.max)
                pb = psg.tile([128, 1], F32, tag="bc", name="bc")
                nc.tensor.matmul(pb[:, :], ones_row[:, :], mxs[:, :],
                                 start=True, stop=True)
                pd = gp.tile([128, 1], F32, tag="pd", name="pd")
                nc.scalar.copy(pd[:, :], pb[:, :])
                nc.vector.reciprocal(ipb[:, :], pd[:, :])
                nc.vector.tensor_mul(naip[:, :], ipb[:, :], naH[:, :])

            # both branches side by side in one wide tile
            pw2 = ep.tile([128, 2 * NT * OUT], F32, tag="pw2", name="pw2")
            nc.vector.scalar_tensor_tensor(pw2[:, 0:384], z1[:, :],
                                           ipb[:, :], cpos[:, :],
                                           op0=OP.mult, op1=OP.add)
            nc.vector.scalar_tensor_tensor(pw2[:, 384:768], z1[:, :],
                                           naip[:, :], cneg[:, :],
                                           op0=OP.mult, op1=OP.add)
            rmx2 = ep.tile([128, 2 * NT], F32, tag="rmx2", name="rmx2")
            nc.vector.tensor_reduce(
                rmx2[:, :],
                pw2[:, :].rearrange("p (g o) -> p g o", o=OUT),
                axis=AX.X, op=OP.max, negate=True)
            ex2 = ep.tile([128, 2 * NT * OUT], F32, tag="ex2", name="ex2")
            for i in range(2 * NT):
                nc.scalar.activation(ex2[:, esl(i)], pw2[:, esl(i)],
                                     AF.Exp, bias=rmx2[:, i:i + 1],
                                     scale=1.0)
            ssum2 = ep.tile([128, 2 * NT], F32, tag="ssum2", name="ssum2")
            nc.vector.tensor_reduce(
                ssum2[:, :],
                ex2[:, :].rearrange("p (g o) -> p g o", o=OUT),
                axis=AX.X, op=OP.add)
            rs2 = ep.tile([128, 2 * NT], F32, tag="rs2", name="rs2")
            nc.vector.reciprocal(rs2[:, :], ssum2[:, :])
            pp_w = ep.tile([128, NT * OUT], F32, tag="pp_w", name="pp_w")
            pn_w = ep.tile([128, NT * OUT], F32, tag="pn_w", name="pn_w")
            for i in range(NT):
                nc.vector.tensor_scalar(pp_w[:, esl(i)], ex2[:, esl(i)],
                                        rs2[:, i:i + 1], None, OP.mult)
                nc.vector.tensor_scalar(pn_w[:, esl(i)],
                                        ex2[:, 384 + OUT * i:384 + OUT *
                                            (i + 1)],
                                        rs2[:, NT + i:NT + i + 1], None,
                                        OP.mult)
            dd = ep.tile([128, NT * OUT], F32, tag="dd", name="dd")
            nc.vector.tensor_sub(dd[:, :], pp_w[:, :], pn_w[:, :])
            msg = ep.tile([128, NT * OUT], F32, tag="msg", name="msg")
            nc.vector.scalar_tensor_tensor(msg[:, :], dd[:, :], 0.5,
                                           ehalf[:, :], op0=OP.mult,
                                           op1=OP.add)
            nc.sync.dma_start(
                out=out_msg[:, :].rearrange("(a p) c -> p a c", p=128),
                in_=msg[:, :].rearrange("p (a c) -> p a c", c=OUT))

    nc.compile()
    return nc


def _get_nc():
    if "nc" not in _CACHE:
        _CACHE["nc"] = _build()
    return _CACHE["nc"]


def _make_in_maps(inputs):
    f = np.float32
    h = np.float16
    sm = np.ascontiguousarray(inputs["smoothed_feature"], dtype=h)
    ori = np.ascontiguousarray(inputs["ori_feature"], dtype=h)
    shared = {
        "Wh016": np.ascontiguousarray(inputs["W_hete0"], dtype=h),
        "Ws016": np.ascontiguousarray(inputs["W_smooth0"], dtype=h),
        "Wl016": np.ascontiguousarray(inputs["W_local0"], dtype=h),
        "Wh116": np.ascontiguousarray(inputs["W_hete1"], dtype=h),
        "Ws116": np.ascontiguousarray(inputs["W_smooth1"], dtype=h),
        "Wl116": np.ascontiguousarray(inputs["W_local1"], dtype=h),
        "b_hete0": np.ascontiguousarray(inputs["b_hete0"], dtype=f),
        "b_hete1": np.ascontiguousarray(inputs["b_hete1"], dtype=f),
        "b_smooth0": np.ascontiguousarray(inputs["b_smooth0"], dtype=f),
        "b_smooth1": np.ascontiguousarray(inputs["b_smooth1"], dtype=f),
        "b_local0": np.ascontiguousarray(inputs["b_local0"], dtype=f),
        "b_local1": np.ascontiguousarray(inputs["b_local1"], dtype=f),
        "prelu_model": np.ascontiguousarray(inputs["prelu_model"], dtype=f),
        "prelu_hete": np.ascontiguousarray(inputs["prelu_hete"], dtype=f),
        "ident": np.eye(128, dtype=f),
        "ident16": np.eye(128, dtype=h),
        "ones_row": np.ones((1, 128), dtype=f),
        "ones_col": np.ones((128, 1), dtype=f),
    }
    in_maps = []
    for r in range(NCORES):
        cm = np.ones((1, NSLOT), dtype=f)
        if r < 7:
            cm[0, r] = 0.0  # drop the raw local-block slot
        # rank 7's own block is k=7, which has no raw slot at all
        m = dict(shared)
        m["x_sm16"] = np.ascontiguousarray(sm[P * r:P * (r + 1)])
        m["x_ori16"] = np.ascontiguousarray(ori[P * r:P * (r + 1)])
        m["colmask"] = cm
        in_maps.append(m)
    return in_maps


def _ensure_ntff_hook():
    """The agent image's antenv lacks axon_hooks; shim it so
    run_bass_kernel_spmd(trace=True) can capture NTFF profiles."""
    if "antenv.axon_hooks" in sys.modules:
        return
    import types
    import antenv
    mod = types.ModuleType("antenv.axon_hooks")
    state = {"hook": None}
    mod.set_axon_ntff_profile_hook = lambda h: state.__setitem__("hook", h)
    mod.get_axon_ntff_profile_hook = lambda: state["hook"]
    sys.modules["antenv.axon_hooks"] = mod
    antenv.axon_hooks = mod
    try:
        from trn_agent_boot.trn_boot import _ntff_profile_via_ctypes
        mod.set_axon_ntff_profile_hook(
            _ntff_profile_via_ctypes("/opt/axon/libaxon_pjrt.so"))
    except Exception as e:
        print(f"ntff hook install failed: {e}", file=sys.stderr)


def run(inputs, trace=False):
    if trace:
        _ensure_ntff_hook()
    nc = _get_nc()
    in_maps = _make_in_maps(inputs)
    res = run_bass_kernel_spmd(nc, in_maps, list(range(NCORES)), trace=trace)
    outs = res.results
    o1 = np.concatenate([outs[r]["out_ori"] for r in range(NCORES)], axis=0)
    o2 = np.concatenate([outs[r]["out_smooth"] for r in range(NCORES)], axis=0)
    o3 = np.concatenate([outs[r]["out_msg"] for r in range(NCORES)], axis=0)
    return (o1.astype(np.float32), o2.astype(np.float32),
            o3.astype(np.float32)), res


def kernel(**inputs):
    (o1, o2, o3), _ = run(inputs, trace=False)
    return (o1, o2, o3)


# revision 78
# speedup vs baseline: 1.0341x; 1.0341x over previous
"""AdaFGL Bass kernel for 8 TRN2 NeuronCores.

Row-shards the N=6144 nodes across 8 cores (768 rows each). The dense
[N,N] relation matrix never touches HBM or SBUF: each core computes its
transposed column-block re^T[j, i_local] = t_full[j] . t_local[i] tile
by tile in PSUM (single-fp16 t, zero-padded to contract=128 so FWL
stays enabled) and immediately converts each tile to q = relu(re-m)
(fp16, scalar engine) and sigma = (q > 0) (fp16 0/1 step via is_gt on
the vector engine at 2x rate), which feed fp16 accumulation matmuls
(q@emb)^T and (sigma@emb)^T sharing one stationary emb chunk. The mean
m is available BEFORE the relation pass via the rank-1 identity
sum(re) = U.U - N with U = colsum(t), so the whole relation phase is
one fused, PE-dense loop. The block max is kept as a per-pair fp16
running max (tensor_max) so no single long reduce ever stalls the
sigma pipeline.

Signal algebra (a = prelu alpha, ip = 1/(mx-m), im = 1/m):
  pos_w = ip*z1 + a*z2 + (1-a)*emb
  neg_w = -a*ip*z1 + (1-a)*emb - z2
  z1 = qe + (m-1)*emb,  z2 = im*qe + se + emb - im*tw
with qe = (relu(re-m)@emb) incl raw diag, se = (step(re-m)@emb) incl
diag, tw = t(t^T emb) = re0@emb incl diag; all diagonal effects fold
into the emb coefficients (diag(re)=1). The max statistic excludes the
diagonal via 12 recomputed local-block tiles with a -BIG*I suppression
(rank-uniform code; the rank-dependent colmask input zeroes the raw
local-block slot). Because re is symmetric, block (k=7, r) equals
block (r, k=7) computed on core 7, so per-core max slots only cover
k<7 and the [1,1] max AllGather launches after k==6 - its whole round
trip overlaps the final fused chunk; its stats chain uses a gpsimd
partition_all_reduce (ext-isa library pre-warmed at startup).

Two pipelined AllGathers: AG-emb [emb fp16] fires right after the hete
MLP (its mesh hides under the softmax chain), AG-t [t^T fp16 | wr | u]
right after the pack; local-block max tiles + smooth MLP + ori MLP
fill the collective window. u is unpacked first as [8,64] across
partitions and reduced with two tiny PE matmuls so m gates the fused
start by <2us. Inputs are host-cast to fp16 (X, W matrices) so MLP
matmuls and transposes run at 1 cycle/row and input DMA bytes halve;
weight/X loads are batched 3D-AP DMAs spread across the sync/scalar/
gpsimd queues so compute starts ~8us after engine init.
"""

import sys, os
sys.path.insert(0, "/opt/trn_rl_repo")

import numpy as np
from contextlib import ExitStack

from concourse import bass, bacc, tile, mybir, bass_isa
from concourse.bass_utils import run_bass_kernel_spmd

F32 = mybir.dt.float32
F16 = mybir.dt.float16
AX = mybir.AxisListType
OP = mybir.AluOpType
AF = mybir.ActivationFunctionType

N = 6144
NCORES = 8
P = N // NCORES            # 768 rows per core
FEAT = 128
INSM = 512
HID = 256
OUT = 64
NT = P // 128              # 6 row tiles per core
NJ = N // 128              # 48 column chunks
# AG-emb payload: emb16 [128,384]f16 = 24576 f32 words
AGE = 24576
# AG-t payload (f32 words): tT16 | wr | u
OFF_W = 24576              # tT16 [64,768]f16
OFF_U = OFF_W + 4096       # wr [64,64]f32
AGW = OFF_U + 64           # u [1,64]f32
NSLOT = 19                 # 7 fused k-slots + 12 local suppressed
                           # (k=7 covered by peers via symmetry of re)
INV_N2 = 1.0 / float(N * N)
BIG = 1.0e6

_CACHE = {}


def _build():
    nc = bacc.Bacc("TRN2", target_bir_lowering=False, debug=False,
                   num_devices=NCORES)

    def din(name, shape, dt=F32):
        return nc.dram_tensor(name, list(shape), dt, kind="ExternalInput").ap()

    def dout(name, shape):
        return nc.dram_tensor(name, list(shape), F32, kind="ExternalOutput").ap()

    x_sm = din("x_sm16", (P, INSM), F16)
    x_ori = din("x_ori16", (P, FEAT), F16)
    Wh0 = din("Wh016", (INSM, HID), F16)
    Ws0 = din("Ws016", (INSM, HID), F16)
    Wl0 = din("Wl016", (FEAT, HID), F16)
    Wh1 = din("Wh116", (HID, OUT), F16)
    Ws1 = din("Ws116", (HID, OUT), F16)
    Wl1 = din("Wl116", (HID, OUT), F16)
    bh0 = din("b_hete0", (HID,)); bh1 = din("b_hete1", (OUT,))
    bs0 = din("b_smooth0", (HID,)); bs1 = din("b_smooth1", (OUT,))
    bl0 = din("b_local0", (HID,)); bl1 = din("b_local1", (OUT,))
    a_model = din("prelu_model", (1,))
    a_hete = din("prelu_hete", (1,))
    ident_d = din("ident", (128, 128))
    ident16_d = din("ident16", (128, 128), F16)
    ones_row_d = din("ones_row", (1, 128))
    ones_col_d = din("ones_col", (128, 1))
    colmask_d = din("colmask", (1, NSLOT))

    out_ori = dout("out_ori", (P, OUT))
    out_smooth = dout("out_smooth", (P, OUT))
    out_msg = dout("out_msg", (P, OUT))

    age_in = nc.dram_tensor("age_in", [1, AGE], F32).ap()
    age_out = nc.dram_tensor("age_out", [NCORES, AGE], F32,
                             addr_space="Shared").ap()
    ag_in = nc.dram_tensor("ag_in", [1, AGW], F32).ap()
    ag_out = nc.dram_tensor("ag_out", [NCORES, AGW], F32,
                            addr_space="Shared").ap()
    ag2_in = nc.dram_tensor("ag2_in", [1, 1], F32).ap()
    ag2_out = nc.dram_tensor("ag2_out", [NCORES, 1], F32,
                             addr_space="Shared").ap()

    with tile.TileContext(nc) as tc, ExitStack() as ctx:
        cp = ctx.enter_context(tc.tile_pool(name="const", bufs=1))

        # warm the gpsimd ext-isa library for partition_all_reduce (one-time
        # ~7.5us load) while input DMAs stream
        zz = cp.tile([128, 1], F32, tag="zz", name="zz")
        nc.vector.memset(zz[:, :], 0.0)
        zzo = cp.tile([128, 1], F32, tag="zzo", name="zzo")
        nc.gpsimd.partition_all_reduce(zzo[:, :], zz[:, :], channels=128,
                                       reduce_op=bass_isa.ReduceOp.max)

        # ---------- input DMAs: consts + x first, spread across queues ----
        ident16 = cp.tile([128, 128], F16, tag="ident16", name="ident16")
        nc.sync.dma_start(out=ident16[:, :], in_=ident16_d[:, :])
        ones_row = cp.tile([1, 128], F32, tag="ones_row", name="ones_row")
        nc.sync.dma_start(out=ones_row[:, :], in_=ones_row_d[:, :])
        ones_col = cp.tile([128, 1], F32, tag="ones_col", name="ones_col")
        nc.sync.dma_start(out=ones_col[:, :], in_=ones_col_d[:, :])
        asc = cp.tile([1, 2], F32, tag="asc", name="asc")
        nc.sync.dma_start(out=asc[:, 0:1], in_=a_model[0:1])
        nc.sync.dma_start(out=asc[:, 1:2], in_=a_hete[0:1])
        x16 = cp.tile([128, NT * INSM], F16, tag="x16", name="x16")
        x16v = x16[:, :].rearrange("p (i c) -> p i c", c=INSM)
        nc.sync.dma_start(
            out=x16v[:, 0:3, :],
            in_=x_sm[0:384, :].rearrange("(a p) c -> p a c", p=128))
        nc.scalar.dma_start(
            out=x16v[:, 3:6, :],
            in_=x_sm[384:768, :].rearrange("(a p) c -> p a c", p=128))

        # consts on gpsimd queue (must stay short: AG trigger lives here)
        ident = cp.tile([128, 128], F32, tag="ident", name="ident")
        nc.gpsimd.dma_start(out=ident[:, :], in_=ident_d[:, :])
        colmask = cp.tile([1, NSLOT], F32, tag="colmask", name="colmask")
        nc.gpsimd.dma_start(out=colmask[:, :], in_=colmask_d[:, :])

        # weights: batched DMAs on sync/scalar
        Wh0_t = cp.tile([128, 4 * HID], F16, tag="Wh0", name="Wh0")
        nc.sync.dma_start(
            out=Wh0_t[:, :].rearrange("p (a c) -> p a c", c=HID),
            in_=Wh0[:, :].rearrange("(a p) c -> p a c", p=128))
        Ws0_t = cp.tile([128, 4 * HID], F16, tag="Ws0", name="Ws0")
        nc.scalar.dma_start(
            out=Ws0_t[:, :].rearrange("p (a c) -> p a c", c=HID),
            in_=Ws0[:, :].rearrange("(a p) c -> p a c", p=128))
        Wh1_t = cp.tile([128, 2 * OUT], F16, tag="Wh1", name="Wh1")
        nc.sync.dma_start(
            out=Wh1_t[:, :].rearrange("p (a c) -> p a c", c=OUT),
            in_=Wh1[:, :].rearrange("(a p) c -> p a c", p=128))
        Ws1_t = cp.tile([128, 2 * OUT], F16, tag="Ws1", name="Ws1")
        nc.scalar.dma_start(
            out=Ws1_t[:, :].rearrange("p (a c) -> p a c", c=OUT),
            in_=Ws1[:, :].rearrange("(a p) c -> p a c", p=128))
        bh0_t = cp.tile([128, 2], F32, tag="bh0", name="bh0")
        nc.sync.dma_start(out=bh0_t[:, :].rearrange("p a -> p a"),
                          in_=bh0[:].rearrange("(a p) -> p a", p=128))
        bs0_t = cp.tile([128, 2], F32, tag="bs0", name="bs0")
        nc.scalar.dma_start(out=bs0_t[:, :].rearrange("p a -> p a"),
                            in_=bs0[:].rearrange("(a p) -> p a", p=128))
        bh1_r = cp.tile([1, OUT], F32, tag="bh1r", name="bh1r")
        nc.sync.dma_start(out=bh1_r[:, :], in_=bh1[:])
        bs1_r = cp.tile([1, OUT], F32, tag="bs1r", name="bs1r")
        nc.scalar.dma_start(out=bs1_r[:, :], in_=bs1[:])

        # ori branch loads (consumed late, in the AG2 window)
        x_o16 = cp.tile([128, NT * FEAT], F16, tag="xo16", name="xo16")
        nc.scalar.dma_start(
            out=x_o16[:, :].rearrange("p (a c) -> p a c", c=FEAT),
            in_=x_ori[:, :].rearrange("(a p) c -> p a c", p=128))
        Wl0_t = cp.tile([128, HID], F16, tag="Wl0", name="Wl0")
        nc.scalar.dma_start(out=Wl0_t[:, :], in_=Wl0[:, :])
        Wl1_t = cp.tile([128, 2 * OUT], F16, tag="Wl1", name="Wl1")
        nc.scalar.dma_start(
            out=Wl1_t[:, :].rearrange("p (a c) -> p a c", c=OUT),
            in_=Wl1[:, :].rearrange("(a p) c -> p a c", p=128))
        bl0_t = cp.tile([128, 2], F32, tag="bl0", name="bl0")
        nc.scalar.dma_start(out=bl0_t[:, :], in_=bl0[:].rearrange(
            "(a p) -> p a", p=128))
        bl1_r = cp.tile([1, OUT], F32, tag="bl1r", name="bl1r")
        nc.scalar.dma_start(out=bl1_r[:, :], in_=bl1[:])

        nbh0_t = cp.tile([128, 2], F32, tag="nbh0", name="nbh0")
        nc.vector.tensor_scalar(nbh0_t[:, :], bh0_t[:, :], -1.0, None, OP.mult)
        nbs0_t = cp.tile([128, 2], F32, tag="nbs0", name="nbs0")
        nc.vector.tensor_scalar(nbs0_t[:, :], bs0_t[:, :], -1.0, None, OP.mult)
        nbl0_t = cp.tile([128, 2], F32, tag="nbl0", name="nbl0")
        nc.vector.tensor_scalar(nbl0_t[:, :], bl0_t[:, :], -1.0, None, OP.mult)

        # ---------- broadcasts ----------
        aM = cp.tile([128, 1], F32, tag="aM", name="aM")
        aH = cp.tile([128, 1], F32, tag="aH", name="aH")
        naM = cp.tile([128, 1], F32, tag="naM", name="naM")
        naH = cp.tile([128, 1], F32, tag="naH", name="naH")
        bh1_b = cp.tile([128, OUT], F32, tag="bh1", name="bh1")
        bs1_b = cp.tile([128, OUT], F32, tag="bs1", name="bs1")
        bl1_b = cp.tile([128, OUT], F32, tag="bl1", name="bl1")
        with tc.tile_pool(name="ps_bc", bufs=2, space="PSUM") as ps0:
            for src, full, nfull in ((asc[:, 0:1], aM, naM),
                                     (asc[:, 1:2], aH, naH)):
                ps = ps0.tile([128, 1], F32, tag="ps_bc", name="ps_bc")
                nc.tensor.matmul(ps[:, :], ones_row[:, :], src,
                                 start=True, stop=True)
                nc.scalar.copy(full[:, :], ps[:, :])
                nc.vector.tensor_scalar(nfull[:, :], full[:, :], -1.0, None,
                                        OP.mult)
            for row, full in ((bh1_r, bh1_b), (bs1_r, bs1_b),
                              (bl1_r, bl1_b)):
                ps = ps0.tile([128, OUT], F32, tag="ps_b1", name="ps_b1")
                nc.tensor.matmul(ps[:, :], ones_row[:, :], row[:, :],
                                 start=True, stop=True)
                nc.scalar.copy(full[:, :], ps[:, :])

        # ---------- transpose x_sm (fp16, 1 cyc/row) ----------
        XT = [cp.tile([128, P], F16, tag=f"xt{f}", name=f"xt{f}")
              for f in range(4)]
        with tc.tile_pool(name="ps_tr", bufs=4, space="PSUM") as pst:
            for i in range(NT):
                for f in range(4):
                    pt = pst.tile([128, 128], F16, tag="tr", name="tr")
                    nc.tensor.transpose(
                        pt[:, :], x16v[:, i, 128 * f:128 * (f + 1)],
                        ident16[:, :])
                    if f % 2 == 0:
                        nc.scalar.copy(XT[f][:, 128 * i:128 * (i + 1)],
                                       pt[:, :])
                    else:
                        nc.vector.tensor_copy(XT[f][:, 128 * i:128 * (i + 1)],
                                              pt[:, :])

        def esl(i):
            return slice(OUT * i, OUT * (i + 1))

        # ---------- MLP helper (fp16 l1+l2 weights/inputs) ----------
        def mlp(XT_tiles, w0, b0t, nb0t, w1, b1_b, na_b, out_wide, pfx):
            with tc.tile_pool(name=pfx + "_h", bufs=1) as hp, \
                 tc.tile_pool(name=pfx + "_r", bufs=2) as rp, \
                 tc.tile_pool(name=pfx + "_ps1", bufs=2, space="PSUM") as ps1, \
                 tc.tile_pool(name=pfx + "_ps2", bufs=2, space="PSUM") as ps2:
                h_tiles = []
                for hs in range(2):
                    h = hp.tile([128, P], F16, tag=f"h{hs}",
                                name=f"{pfx}h{hs}")
                    for ns2 in range(2):
                        sl = slice(384 * ns2, 384 * (ns2 + 1))
                        pp = ps1.tile([128, 384], F32, tag="l1", name="l1")
                        nf = len(XT_tiles)
                        for fc in range(nf):
                            nc.tensor.matmul(pp[:, :], w0(fc, hs),
                                             XT_tiles[fc][:, sl],
                                             start=(fc == 0),
                                             stop=(fc == nf - 1))
                        r1 = rp.tile([128, 384], F32, tag="r1", name="r1")
                        nc.scalar.activation(r1[:, :], pp[:, :], AF.Relu,
                                             bias=b0t[:, hs:hs + 1],
                                             scale=1.0)
                        r2 = rp.tile([128, 384], F32, tag="r2", name="r2")
                        nc.scalar.activation(r2[:, :], pp[:, :], AF.Relu,
                                             bias=nb0t[:, hs:hs + 1],
                                             scale=-1.0)
                        nc.vector.scalar_tensor_tensor(
                            h[:, sl], r2[:, :], na_b[:, :], r1[:, :],
                            op0=OP.mult, op1=OP.add)
                    h_tiles.append(h)
                for i in range(NT):
                    pp = ps2.tile([128, OUT], F32, tag="l2", name="l2")
                    for hs in range(2):
                        nc.tensor.matmul(pp[:, :],
                                         h_tiles[hs][:, 128 * i:128 * (i + 1)],
                                         w1(hs), start=(hs == 0),
                                         stop=(hs == 1))
                    nc.vector.scalar_tensor_tensor(
                        out_wide[:, esl(i)], pp[:, :], 1.0, b1_b[:, :],
                        op0=OP.mult, op1=OP.add)

        def w0h(fc, hs):
            return Wh0_t[:, HID * fc + 128 * hs:HID * fc + 128 * (hs + 1)]

        def w0s(fc, hs):
            return Ws0_t[:, HID * fc + 128 * hs:HID * fc + 128 * (hs + 1)]

        def w1h(hs):
            return Wh1_t[:, OUT * hs:OUT * (hs + 1)]

        def w1s(hs):
            return Ws1_t[:, OUT * hs:OUT * (hs + 1)]

        # persistent phase-1 outputs
        emb_loc = cp.tile([128, NT * OUT], F32, tag="emb_loc", name="emb_loc")
        emb16_loc = cp.tile([128, NT * OUT], F16, tag="e16l", name="e16l")
        tT16 = cp.tile([128, P], F16, tag="tT16", name="tT16")
        tw = cp.tile([128, NT * OUT], F32, tag="tw", name="tw")
        maxs = cp.tile([128, NSLOT], F32, tag="maxs", name="maxs")
        nc.vector.memset(tT16[64:128, :], 0.0)

        # hete MLP first (feeds the collective)
        mlp(XT, w0h, bh0_t, nbh0_t, w1h, bh1_b, naH, emb_loc, "hete")
        nc.vector.tensor_copy(emb16_loc[:, :], emb_loc[:, :])

        # AG-emb goes out as soon as the hete MLP is done
        nc.sync.dma_start(out=age_in[0:1, 0:AGE],
                          in_=emb16_loc[:, :].bitcast(F32))
        nc.gpsimd.collective_compute(
            "AllGather", OP.bypass,
            ins=[age_in[:, :]],
            outs=[age_out[:, :]],
            replica_groups=[list(range(NCORES))])

        # ---------- softmax -> t = e/||e|| (fp16), wr, u ----------
        wr_sb = cp.tile([64, 64], F32, tag="wr_sb", name="wr_sb")
        u_sb = cp.tile([1, 64], F32, tag="u_sb", name="u_sb")
        with tc.tile_pool(name="smax", bufs=1) as sp, \
             tc.tile_pool(name="ps_wr", bufs=1, space="PSUM") as pswr, \
             tc.tile_pool(name="ps_tr2", bufs=2, space="PSUM") as pst2:
            rmx = sp.tile([128, NT], F32, tag="rmx", name="rmx")
            nc.vector.tensor_reduce(
                rmx[:, :],
                emb_loc[:, :].rearrange("p (g o) -> p g o", o=OUT),
                axis=AX.X, op=OP.max, negate=True)
            ex_w = sp.tile([128, NT * OUT], F32, tag="ex_w", name="ex_w")
            for i in range(NT):
                nc.scalar.activation(ex_w[:, esl(i)], emb_loc[:, esl(i)],
                                     AF.Exp, bias=rmx[:, i:i + 1], scale=1.0)
            sq_w = sp.tile([128, NT * OUT], F32, tag="sq_w", name="sq_w")
            nc.vector.tensor_mul(sq_w[:, :], ex_w[:, :], ex_w[:, :])
            dsum = sp.tile([128, NT], F32, tag="dsum", name="dsum")
            nc.vector.tensor_reduce(
                dsum[:, :],
                sq_w[:, :].rearrange("p (g o) -> p g o", o=OUT),
                axis=AX.X, op=OP.add)
            rd = sp.tile([128, NT], F32, tag="rd", name="rd")
            nc.vector.reciprocal(rd[:, :], dsum[:, :])
            isd = sp.tile([128, NT], F32, tag="isd", name="isd")
            nc.scalar.activation(isd[:, :], rd[:, :], AF.Sqrt)
            t16 = sp.tile([128, NT * OUT], F16, tag="t16", name="t16")
            for i in range(NT):
                nc.vector.tensor_scalar(t16[:, esl(i)], ex_w[:, esl(i)],
                                        isd[:, i:i + 1], None, OP.mult)
            ps_wr = pswr.tile([64, 64], F32, tag="wr", name="pswr")
            for i in range(NT):
                nc.tensor.matmul(ps_wr[:, :], t16[:, esl(i)],
                                 emb16_loc[:, esl(i)],
                                 start=(i == 0), stop=(i == NT - 1))
                pt = pst2.tile([64, 128], F16, tag="ttr", name="ttr")
                nc.tensor.transpose(pt[:, :], t16[:, esl(i)], ident16[:, :])
                nc.scalar.copy(tT16[0:64, 128 * i:128 * (i + 1)], pt[:, :])
            nc.scalar.copy(wr_sb[:, :], ps_wr[:, :])
            # u = colsum(t): rowsum of tT16, transposed to a row
            uT = sp.tile([64, 1], F32, tag="uT", name="uT")
            nc.vector.tensor_reduce(uT[:, :], tT16[0:64, :], axis=AX.X,
                                    op=OP.add)
            pu = pst2.tile([1, 64], F32, tag="put", name="put")
            nc.tensor.transpose(pu[:, :], uT[:, :], ident[0:64, 0:64])
            nc.scalar.copy(u_sb[:, :], pu[:, :])

        # ---------- pack + AG-t ----------
        nc.sync.dma_start(out=ag_in[0:1, 0:OFF_W],
                          in_=tT16[0:64, :].bitcast(F32))
        nc.sync.dma_start(out=ag_in[0:1, OFF_W:OFF_U], in_=wr_sb[:, :])
        nc.scalar.dma_start(out=ag_in[0:1, OFF_U:AGW], in_=u_sb[:, :])
        nc.gpsimd.collective_compute(
            "AllGather", OP.bypass,
            ins=[ag_in[:, :]],
            outs=[ag_out[:, :]],
            replica_groups=[list(range(NCORES))])

        # ---------- AG bubble: local-block max tiles + smooth MLP ----------
        negbigI = cp.tile([128, 128], F32, tag="negbigI", name="negbigI")
        nc.vector.tensor_scalar(negbigI[:, :], ident[:, :], -BIG, None,
                                OP.mult)
        with tc.tile_pool(name="ps_rex", bufs=2, space="PSUM") as psre:
            for s in range(NT):
                for h in range(2):
                    sl = slice(384 * h, 384 * (h + 1))
                    pp = psre.tile([128, 384], F32, tag="rex", name="rex")
                    nc.tensor.matmul(pp[:, :],
                                     tT16[:, 128 * s:128 * (s + 1)],
                                     tT16[:, sl], start=True, stop=True)
                    if (s // 3) == h:
                        off = 128 * s - 384 * h
                        nc.vector.scalar_tensor_tensor(
                            pp[:, off:off + 128], ident[:, :], -BIG,
                            pp[:, off:off + 128], op0=OP.mult, op1=OP.add)
                    slot = 7 + 2 * s + h
                    nc.vector.tensor_reduce(maxs[:, slot:slot + 1], pp[:, :],
                                            axis=AX.X, op=OP.max)
        with tc.tile_pool(name="mlpout", bufs=1) as mo:
            sm_out = mo.tile([128, NT * OUT], F32, tag="smo", name="smo")
            mlp(XT, w0s, bs0_t, nbs0_t, w1s, bs1_b, naM, sm_out, "smooth")
            nc.sync.dma_start(
                out=out_smooth[:, :].rearrange("(a p) c -> p a c", p=128),
                in_=sm_out[:, :].rearrange("p (a c) -> p a c", c=OUT))

        # ori MLP also fills the AG window
        XTo = cp.tile([128, P], F16, tag="xto", name="xto")
        with tc.tile_pool(name="ps_or", bufs=2, space="PSUM") as pso:
            for i in range(NT):
                pt = pso.tile([128, 128], F16, tag="otr", name="otr")
                nc.tensor.transpose(
                    pt[:, :],
                    x_o16[:, :].rearrange("p (a c) -> p a c",
                                          c=FEAT)[:, i, :],
                    ident16[:, :])
                nc.scalar.copy(XTo[:, 128 * i:128 * (i + 1)], pt[:, :])

        def w0l(fc, hs):
            return Wl0_t[:, 128 * hs:128 * (hs + 1)]

        def w1l(hs):
            return Wl1_t[:, OUT * hs:OUT * (hs + 1)]

        with tc.tile_pool(name="mlpout2", bufs=1) as mo2:
            or_out = mo2.tile([128, NT * OUT], F32, tag="oro", name="oro")
            mlp([XTo], w0l, bl0_t, nbl0_t, w1l, bl1_b, naM, or_out, "ori")
            nc.sync.dma_start(
                out=out_ori[:, :].rearrange("(a p) c -> p a c", p=128),
                in_=or_out[:, :].rearrange("p (a c) -> p a c", c=OUT))

        # ---------- unpack AG-emb + AG-t ----------
        tf16 = [cp.tile([128, P], F16, tag=f"tf{k}", name=f"tf{k}")
                for k in range(NCORES)]
        emb16 = [cp.tile([128, NT * OUT], F16, tag=f"e16_{k}",
                         name=f"e16_{k}") for k in range(NCORES)]
        w16 = cp.tile([64, 64], F16, tag="w16", name="w16")
        mb = cp.tile([128, 1], F32, tag="mb", name="mb")
        nmb = cp.tile([128, 1], F32, tag="nmb", name="nmb")
        # u gather FIRST: it unlocks the m chain / fused start.
        # uall8 [8,64] across partitions; U via two tiny PE matmuls.
        with tc.tile_pool(name="unpack", bufs=1) as up, \
             tc.tile_pool(name="ps_m", bufs=2, space="PSUM") as psm:
            uall8 = up.tile([8, 64], F32, tag="uall8", name="uall8")
            nc.sync.dma_start(out=uall8[:, :], in_=ag_out[:, OFF_U:AGW])
            pU = psm.tile([64, 1], F32, tag="pU", name="pU")
            nc.tensor.matmul(pU[:, :], uall8[:, :], ones_col[0:8, :],
                             start=True, stop=True)
            UT = up.tile([64, 1], F32, tag="UT", name="UT")
            nc.scalar.copy(UT[:, :], pU[:, :])
            puu = psm.tile([1, 1], F32, tag="puu", name="puu")
            nc.tensor.matmul(puu[:, :], UT[:, :], UT[:, :],
                             start=True, stop=True)
            m01 = up.tile([1, 1], F32, tag="m01", name="m01")
            nc.vector.tensor_scalar(m01[:, :], puu[:, :], -float(N), INV_N2,
                                    OP.add, OP.mult)
            pb = psm.tile([128, 1], F32, tag="mbc", name="mbc")
            nc.tensor.matmul(pb[:, :], ones_row[:, :], m01[:, :],
                             start=True, stop=True)
            nc.scalar.copy(mb[:, :], pb[:, :])
            nc.vector.tensor_scalar(nmb[:, :], mb[:, :], -1.0, None, OP.mult)
            # big per-core unpacks (k-ascending: the fused loop chases them)
            for k in range(NCORES):
                nc.vector.memset(tf16[k][64:128, :], 0.0)
                eng = nc.sync if k % 2 == 0 else nc.scalar
                eng2 = nc.scalar if k % 2 == 0 else nc.sync
                eng.dma_start(out=tf16[k][0:64, :].bitcast(F32),
                              in_=ag_out[k:k + 1, 0:OFF_W])
                eng2.dma_start(out=emb16[k][:, :].bitcast(F32),
                               in_=age_out[k:k + 1, 0:AGE])
            # wr gather (needed only in the epilogue)
            wrall = up.tile([64, 8 * 64], F32, tag="wrall", name="wrall")
            nc.scalar.dma_start(
                out=wrall[:, :].rearrange("p (k c) -> p k c", k=NCORES),
                in_=ag_out[:, OFF_W:OFF_U].rearrange("k (p c) -> p k c",
                                                     p=64))
            w_sb = up.tile([64, 64], F32, tag="w_sb", name="w_sb")
            nc.vector.tensor_reduce(
                w_sb[:, :],
                wrall[:, :].rearrange("p (k c) -> p c k", k=NCORES),
                axis=AX.X, op=OP.add)
            nc.vector.tensor_copy(w16[:, :], w_sb[:, :])

        # epilogue scalars that only need m / aH
        nimb = cp.tile([128, 1], F32, tag="nimb", name="nimb")  # -1/m
        imb = cp.tile([128, 1], F32, tag="imb", name="imb")     # 1/m
        omaH = cp.tile([128, 1], F32, tag="omaH", name="omaH")  # 1-aH
        n1m = cp.tile([128, 1], F32, tag="n1m", name="n1m")     # m-1
        nc.vector.reciprocal(imb[:, :], mb[:, :])
        nc.vector.tensor_scalar(nimb[:, :], imb[:, :], -1.0, None, OP.mult)
        nc.vector.tensor_scalar(omaH[:, :], aH[:, :], -1.0, 1.0, OP.mult,
                                OP.add)
        nc.vector.tensor_scalar(n1m[:, :], mb[:, :], 1.0, -1.0, OP.mult,
                                OP.add)

        # ---------- fused relation + propagation pass ----------
        qeT = cp.tile([64, P], F32, tag="qeT", name="qeT")
        seT = cp.tile([64, P], F32, tag="seT", name="seT")
        with tc.tile_pool(name="ps_re", bufs=3, space="PSUM") as psre, \
             tc.tile_pool(name="ps_acc", bufs=1, space="PSUM") as pacc, \
             tc.tile_pool(name="qc", bufs=3) as qcp, \
             tc.tile_pool(name="sg", bufs=3) as sgp, \
             tc.tile_pool(name="rmp", bufs=2) as rmp:
            qe_ps = [pacc.tile([64, 384], F32, tag=f"qe{h}", name=f"qe{h}")
                     for h in range(2)]
            se_ps = [pacc.tile([64, 384], F32, tag=f"se{h}", name=f"se{h}")
                     for h in range(2)]
            for k in range(NCORES):
                rm = rmp.tile([128, P], F16, tag="rm", name="rm")
                for sub in range(NT):
                    j = NT * k + sub
                    q2 = qcp.tile([128, P], F16, tag="q2", name="q2")
                    s2 = sgp.tile([128, P], F16, tag="s2", name="s2")
                    pps = []
                    for h in range(2):
                        pp = psre.tile([128, 384], F32, tag="rem",
                                       name="rem")
                        nc.tensor.matmul(
                            pp[:, :],
                            tf16[k][:, 128 * sub:128 * (sub + 1)],
                            tT16[:, 384 * h:384 * (h + 1)],
                            start=True, stop=True)
                        pps.append(pp)
                    for h in range(2):
                        qsl = slice(384 * h, 384 * (h + 1))
                        nc.scalar.activation(q2[:, qsl], pps[h][:, :],
                                             AF.Relu, bias=nmb[:, :],
                                             scale=1.0)
                    nc.vector.tensor_scalar(s2[:, :], q2[:, :], 0.0,
                                            None, OP.is_gt)
                    if k < 7:
                        if sub == 0:
                            nc.vector.tensor_scalar(rm[:, :], q2[:, :],
                                                    0.0, None, OP.max)
                        else:
                            nc.vector.tensor_max(rm[:, :], rm[:, :],
                                                 q2[:, :])
                    for h in range(2):
                        qsl = slice(384 * h, 384 * (h + 1))
                        nc.tensor.matmul(qe_ps[h][:, :],
                                         emb16[k][:, esl(sub)], q2[:, qsl],
                                         start=(j == 0), stop=(j == NJ - 1),
                                         skip_group_check=True)
                        nc.tensor.matmul(se_ps[h][:, :],
                                         emb16[k][:, esl(sub)], s2[:, qsl],
                                         start=(j == 0), stop=(j == NJ - 1),
                                         skip_group_check=True)
                if k < 7:
                    nc.vector.tensor_reduce(maxs[:, k:k + 1], rm[:, :],
                                            axis=AX.X, op=OP.max)
                if k == 6:
                    # k=7 slot is covered by peers (re is symmetric):
                    # stats + AG2 launch overlap the last fused chunk
                    nc.vector.tensor_scalar(maxs[:, 7:NSLOT],
                                            maxs[:, 7:NSLOT], mb[:, :],
                                            0.0, OP.subtract, OP.max)
                    mall = cp.tile([128, NSLOT], F32, tag="mall",
                                   name="mall")
                    nc.gpsimd.partition_all_reduce(
                        mall[:, :], maxs[:, :], channels=128,
                        reduce_op=bass_isa.ReduceOp.max)
                    mrow = cp.tile([1, NSLOT], F32, tag="mrow", name="mrow")
                    nc.vector.tensor_mul(mrow[:, :], mall[0:1, :],
                                         colmask[:, :])
                    mx01 = cp.tile([1, 1], F32, tag="mx01", name="mx01")
                    nc.vector.tensor_reduce(mx01[:, :], mrow[:, :],
                                            axis=AX.X, op=OP.max)
                    nc.sync.dma_start(out=ag2_in[:, :], in_=mx01[:, :])
                    nc.gpsimd.collective_compute(
                        "AllGather", OP.bypass,
                        ins=[ag2_in[:, :]],
                        outs=[ag2_out[:, :]],
                        replica_groups=[list(range(NCORES))])
            for h in range(2):
                sl = slice(384 * h, 384 * (h + 1))
                nc.scalar.copy(qeT[:, sl], qe_ps[h][:, :])
                nc.scalar.copy(seT[:, sl], se_ps[h][:, :])

        # ---------- epilogue prep (independent of qmax) ----------
        with tc.tile_pool(name="epi", bufs=1) as ep, \
             tc.tile_pool(name="ps_epi", bufs=2, space="PSUM") as pse:
            qe_nm = ep.tile([128, NT * OUT], F32, tag="qe_nm", name="qe_nm")
            se_nm = ep.tile([128, NT * OUT], F32, tag="se_nm", name="se_nm")
            for i in range(NT):
                pp = pse.tile([128, OUT], F32, tag="twp", name="twp")
                nc.tensor.matmul(pp[:, :], tT16[0:64, 128 * i:128 * (i + 1)],
                                 w16[:, :], start=True, stop=True)
                nc.scalar.copy(tw[:, esl(i)], pp[:, :])
            for i in range(NT):
                csl = slice(128 * i, 128 * (i + 1))
                pq = pse.tile([128, 64], F32, tag="tq", name="tq")
                nc.tensor.transpose(pq[:, :], qeT[:, csl], ident[0:64, 0:64])
                nc.scalar.copy(qe_nm[:, esl(i)], pq[:, :])
                pc = pse.tile([128, 64], F32, tag="tc", name="tc")
                nc.tensor.transpose(pc[:, :], seT[:, csl], ident[0:64, 0:64])
                nc.scalar.copy(se_nm[:, esl(i)], pc[:, :])

            z1 = ep.tile([128, NT * OUT], F32, tag="z1", name="z1")
            nc.vector.scalar_tensor_tensor(z1[:, :], emb_loc[:, :], n1m[:, :],
                                           qe_nm[:, :], op0=OP.mult,
                                           op1=OP.add)
            z2 = ep.tile([128, NT * OUT], F32, tag="z2", name="z2")
            nc.vector.tensor_scalar(z2[:, :], qe_nm[:, :], imb[:, :], None,
                                    OP.mult)
            nc.vector.tensor_add(z2[:, :], z2[:, :], se_nm[:, :])
            nc.vector.scalar_tensor_tensor(z2[:, :], tw[:, :], nimb[:, :],
                                           z2[:, :], op0=OP.mult, op1=OP.add)
            nc.vector.tensor_add(z2[:, :], z2[:, :], emb_loc[:, :])
            t2 = ep.tile([128, NT * OUT], F32, tag="t2", name="t2")
            nc.vector.tensor_scalar(t2[:, :], emb_loc[:, :], omaH[:, :],
                                    None, OP.mult)
            cpos = ep.tile([128, NT * OUT], F32, tag="cpos", name="cpos")
            nc.vector.scalar_tensor_tensor(cpos[:, :], z2[:, :], aH[:, :],
                                           t2[:, :], op0=OP.mult, op1=OP.add)
            cneg = ep.tile([128, NT * OUT], F32, tag="cneg", name="cneg")
            nc.vector.tensor_sub(cneg[:, :], t2[:, :], z2[:, :])
            ehalf = ep.tile([128, NT * OUT], F32, tag="ehalf", name="ehalf")
            nc.vector.tensor_scalar(ehalf[:, :], emb_loc[:, :], 0.5, None,
                                    OP.mult)

            # ---------- qmax -> ip ----------
            ipb = cp.tile([128, 1], F32, tag="ipb", name="ipb")
            naip = cp.tile([128, 1], F32, tag="naip", name="naip")
            with tc.tile_pool(name="glob", bufs=1) as gp, \
                 tc.tile_pool(name="ps_gl", bufs=1, space="PSUM") as psg:
                m8 = gp.tile([1, 8], F32, tag="m8", name="m8")
                nc.sync.dma_start(out=m8[:, :], in_=ag2_out[:, 0:1])
                mxs = gp.tile([1, 1], F32, tag="mxs", name="mxs")
                nc.vector.tensor_reduce(mxs[:, :], m8[:, :], axis=AX.X,
                                        op=OP.max)
                pb = psg.tile([128, 1], F32, tag="bc", name="bc")
                nc.tensor.matmul(pb[:, :], ones_row[:, :], mxs[:, :],
                                 start=True, stop=True)
                pd = gp.tile([128, 1], F32, tag="pd", name="pd")
                nc.scalar.copy(pd[:, :], pb[:, :])
                nc.vector.reciprocal(ipb[:, :], pd[:, :])
                nc.vector.tensor_mul(naip[:, :], ipb[:, :], naH[:, :])

            # both branches side by side in one wide tile
            pw2 = ep.tile([128, 2 * NT * OUT], F32, tag="pw2", name="pw2")
            nc.vector.scalar_tensor_tensor(pw2[:, 0:384], z1[:, :],
                                           ipb[:, :], cpos[:, :],
                                           op0=OP.mult, op1=OP.add)
            nc.vector.scalar_tensor_tensor(pw2[:, 384:768], z1[:, :],
                                           naip[:, :], cneg[:, :],
                                           op0=OP.mult, op1=OP.add)
            rmx2 = ep.tile([128, 2 * NT], F32, tag="rmx2", name="rmx2")
            nc.vector.tensor_reduce(
                rmx2[:, :],
                pw2[:, :].rearrange("p (g o) -> p g o", o=OUT),
                axis=AX.X, op=OP.max, negate=True)
            ex2 = ep.tile([128, 2 * NT * OUT], F32, tag="ex2", name="ex2")
            for i in range(2 * NT):
                nc.scalar.activation(ex2[:, esl(i)], pw2[:, esl(i)],
                                     AF.Exp, bias=rmx2[:, i:i + 1],
                                     scale=1.0)
            ssum2 = ep.tile([128, 2 * NT], F32, tag="ssum2", name="ssum2")
            nc.vector.tensor_reduce(
                ssum2[:, :],
                ex2[:, :].rearrange("p (g o) -> p g o", o=OUT),
                axis=AX.X, op=OP.add)
            rs2 = ep.tile([128, 2 * NT], F32, tag="rs2", name="rs2")
            nc.vector.reciprocal(rs2[:, :], ssum2[:, :])
            pp_w = ep.tile([128, NT * OUT], F32, tag="pp_w", name="pp_w")
            pn_w = ep.tile([128, NT * OUT], F32, tag="pn_w", name="pn_w")
            for i in range(NT):
                nc.vector.tensor_scalar(pp_w[:, esl(i)], ex2[:, esl(i)],
                                        rs2[:, i:i + 1], None, OP.mult)
                nc.vector.tensor_scalar(pn_w[:, esl(i)],
                                        ex2[:, 384 + OUT * i:384 + OUT *
                                            (i + 1)],
                                        rs2[:, NT + i:NT + i + 1], None,
                                        OP.mult)
            dd = ep.tile([128, NT * OUT], F32, tag="dd", name="dd")
            nc.vector.tensor_sub(dd[:, :], pp_w[:, :], pn_w[:, :])
            msg = ep.tile([128, NT * OUT], F32, tag="msg", name="msg")
            nc.vector.scalar_tensor_tensor(msg[:, :], dd[:, :], 0.5,
                                           ehalf[:, :], op0=OP.mult,
                                           op1=OP.add)
            nc.sync.dma_start(
                out=out_msg[:, :].rearrange("(a p) c -> p a c", p=128),
                in_=msg[:, :].rearrange("p (a c) -> p a c", c=OUT))

    nc.compile()
    return nc


def _get_nc():
    if "nc" not in _CACHE:
        _CACHE["nc"] = _build()
    return _CACHE["nc"]


def _make_in_maps(inputs):
    f = np.float32
    h = np.float16
    sm = np.ascontiguousarray(inputs["smoothed_feature"], dtype=h)
    ori = np.ascontiguousarray(inputs["ori_feature"], dtype=h)
    shared = {
        "Wh016": np.ascontiguousarray(inputs["W_hete0"], dtype=h),
        "Ws016": np.ascontiguousarray(inputs["W_smooth0"], dtype=h),
        "Wl016": np.ascontiguousarray(inputs["W_local0"], dtype=h),
        "Wh116": np.ascontiguousarray(inputs["W_hete1"], dtype=h),
        "Ws116": np.ascontiguousarray(inputs["W_smooth1"], dtype=h),
        "Wl116": np.ascontiguousarray(inputs["W_local1"], dtype=h),
        "b_hete0": np.ascontiguousarray(inputs["b_hete0"], dtype=f),
        "b_hete1": np.ascontiguousarray(inputs["b_hete1"], dtype=f),
        "b_smooth0": np.ascontiguousarray(inputs["b_smooth0"], dtype=f),
        "b_smooth1": np.ascontiguousarray(inputs["b_smooth1"], dtype=f),
        "b_local0": np.ascontiguousarray(inputs["b_local0"], dtype=f),
        "b_local1": np.ascontiguousarray(inputs["b_local1"], dtype=f),
        "prelu_model": np.ascontiguousarray(inputs["prelu_model"], dtype=f),
        "prelu_hete": np.ascontiguousarray(inputs["prelu_hete"], dtype=f),
        "ident": np.eye(128, dtype=f),
        "ident16": np.eye(128, dtype=h),
        "ones_row": np.ones((1, 128), dtype=f),
        "ones_col": np.ones((128, 1), dtype=f),
    }
    in_maps = []
    for r in range(NCORES):
        cm = np.ones((1, NSLOT), dtype=f)
        if r < 7:
            cm[0, r] = 0.0  # drop the raw local-block slot
        # rank 7's own block is k=7, which has no raw slot at all
        m = dict(shared)
        m["x_sm16"] = np.ascontiguousarray(sm[P * r:P * (r + 1)])
        m["x_ori16"] = np.ascontiguousarray(ori[P * r:P * (r + 1)])
        m["colmask"] = cm
        in_maps.append(m)
    return in_maps


def _ensure_ntff_hook():
    """The agent image's antenv lacks axon_hooks; shim it so
    run_bass_kernel_spmd(trace=True) can capture NTFF profiles."""
    if "antenv.axon_hooks" in sys.modules:
        return
    import types
    import antenv
    mod = types.ModuleType("antenv.axon_hooks")
    state = {"hook": None}
    mod.set_axon_ntff_profile_hook = lambda h: state.__setitem__("hook", h)
    mod.get_axon_ntff_profile_hook = lambda: state["hook"]
    sys.modules["antenv.axon_hooks"] = mod
    antenv.axon_hooks = mod
    try:
        from trn_agent_boot.trn_boot import _ntff_profile_via_ctypes
        mod.set_axon_ntff_profile_hook(
            _ntff_profile_via_ctypes("/opt/axon/libaxon_pjrt.so"))
    except Exception as e:
        print(f"ntff hook install failed: {e}", file=sys.stderr)


def run(inputs, trace=False):
    if trace:
        _ensure_ntff_hook()
    nc = _get_nc()
    in_maps = _make_in_maps(inputs)
    res = run_bass_kernel_spmd(nc, in_maps, list(range(NCORES)), trace=trace)
    outs = res.results
    o1 = np.concatenate([outs[r]["out_ori"] for r in range(NCORES)], axis=0)
    o2 = np.concatenate([outs[r]["out_smooth"] for r in range(NCORES)], axis=0)
    o3 = np.concatenate([outs[r]["out_msg"] for r in range(NCORES)], axis=0)
    return (o1.astype(np.float32), o2.astype(np.float32),
            o3.astype(np.float32)), res


def kernel(**inputs):
    (o1, o2, o3), _ = run(inputs, trace=False)
    return (o1, o2, o3)


# revision 79
# speedup vs baseline: 1.0668x; 1.0316x over previous
"""AdaFGL Bass kernel for 8 TRN2 NeuronCores.

Row-shards the N=6144 nodes across 8 cores (768 rows each). The dense
[N,N] relation matrix never touches HBM or SBUF: each core computes its
transposed column-block re^T[j, i_local] = t_full[j] . t_local[i] tile
by tile in PSUM (single-fp16 t, zero-padded to contract=128 so FWL
stays enabled) and immediately converts each tile to q = relu(re-m)
(fp16, scalar engine) and sigma = (q > 0) (fp16 0/1 step via is_gt on
the vector engine at 2x rate), which feed fp16 accumulation matmuls
(q@emb)^T and (sigma@emb)^T sharing one stationary emb chunk. The mean
m is available BEFORE the relation pass via the rank-1 identity
sum(re) = U.U - N with U = colsum(t), so the whole relation phase is
one fused, PE-dense loop. The block max is kept as a per-pair fp16
running max (tensor_max) so no single long reduce ever stalls the
sigma pipeline.

Signal algebra (a = prelu alpha, ip = 1/(mx-m), im = 1/m):
  pos_w = ip*z1 + a*z2 + (1-a)*emb
  neg_w = -a*ip*z1 + (1-a)*emb - z2
  z1 = qe + (m-1)*emb,  z2 = im*qe + se + emb - im*tw
with qe = (relu(re-m)@emb) incl raw diag, se = (step(re-m)@emb) incl
diag, tw = t(t^T emb) = re0@emb incl diag; all diagonal effects fold
into the emb coefficients (diag(re)=1). The max statistic excludes the
diagonal via 12 recomputed local-block tiles with a -BIG*I suppression
(rank-uniform code; the rank-dependent colmask input zeroes the raw
local-block slot). Because re is symmetric, block (k=7, r) equals
block (r, k=7) computed on core 7, so per-core max slots only cover
k<7 and the [1,1] max AllGather launches after k==6 - its whole round
trip overlaps the final fused chunk; its stats chain uses a gpsimd
partition_all_reduce (ext-isa library pre-warmed at startup).

Two pipelined AllGathers: AG-emb [emb fp16] fires right after the hete
MLP (its mesh hides under the softmax chain), AG-t [t^T fp16 | wr | u]
right after the pack; local-block max tiles + smooth MLP + ori MLP
fill the collective window. u is unpacked first as [8,64] across
partitions and reduced with two tiny PE matmuls so m gates the fused
start by <2us. Inputs are host-cast to fp16 (X, W matrices) so MLP
matmuls and transposes run at 1 cycle/row and input DMA bytes halve;
weight/X loads are batched 3D-AP DMAs spread across the sync/scalar/
gpsimd queues so compute starts ~8us after engine init.
"""

import sys, os
sys.path.insert(0, "/opt/trn_rl_repo")

import numpy as np
from contextlib import ExitStack

from concourse import bass, bacc, tile, mybir, bass_isa
from concourse.bass_utils import run_bass_kernel_spmd

F32 = mybir.dt.float32
F16 = mybir.dt.float16
AX = mybir.AxisListType
OP = mybir.AluOpType
AF = mybir.ActivationFunctionType

N = 6144
NCORES = 8
P = N // NCORES            # 768 rows per core
FEAT = 128
INSM = 512
HID = 256
OUT = 64
NT = P // 128              # 6 row tiles per core
NJ = N // 128              # 48 column chunks
# AG-emb payload: emb16 [128,384]f16 = 24576 f32 words
AGE = 24576
# AG-t payload (f32 words): tT16 | wr | u
OFF_W = 24576              # tT16 [64,768]f16
OFF_U = OFF_W + 4096       # wr [64,64]f32
AGW = OFF_U + 64           # u [1,64]f32
NSLOT = 19                 # 7 fused k-slots + 12 local suppressed
                           # (k=7 covered by peers via symmetry of re)
INV_N2 = 1.0 / float(N * N)
BIG = 1.0e6

_CACHE = {}


def _build():
    nc = bacc.Bacc("TRN2", target_bir_lowering=False, debug=False,
                   num_devices=NCORES)

    def din(name, shape, dt=F32):
        return nc.dram_tensor(name, list(shape), dt, kind="ExternalInput").ap()

    def dout(name, shape):
        return nc.dram_tensor(name, list(shape), F32, kind="ExternalOutput").ap()

    x_sm = din("x_sm16", (P, INSM), F16)
    x_ori = din("x_ori16", (P, FEAT), F16)
    Wh0 = din("Wh016", (INSM, HID), F16)
    Ws0 = din("Ws016", (INSM, HID), F16)
    Wl0 = din("Wl016", (FEAT, HID), F16)
    Wh1 = din("Wh116", (HID, OUT), F16)
    Ws1 = din("Ws116", (HID, OUT), F16)
    Wl1 = din("Wl116", (HID, OUT), F16)
    bh0 = din("b_hete0", (HID,)); bh1 = din("b_hete1", (OUT,))
    bs0 = din("b_smooth0", (HID,)); bs1 = din("b_smooth1", (OUT,))
    bl0 = din("b_local0", (HID,)); bl1 = din("b_local1", (OUT,))
    a_model = din("prelu_model", (1,))
    a_hete = din("prelu_hete", (1,))
    ident_d = din("ident", (128, 128))
    ident16_d = din("ident16", (128, 128), F16)
    ones_row_d = din("ones_row", (1, 128))
    ones_col_d = din("ones_col", (128, 1))
    colmask_d = din("colmask", (1, NSLOT))

    out_ori = dout("out_ori", (P, OUT))
    out_smooth = dout("out_smooth", (P, OUT))
    out_msg = dout("out_msg", (P, OUT))

    age_in = nc.dram_tensor("age_in", [1, AGE], F32).ap()
    age_out = nc.dram_tensor("age_out", [NCORES, AGE], F32,
                             addr_space="Shared").ap()
    ag_in = nc.dram_tensor("ag_in", [1, AGW], F32).ap()
    ag_out = nc.dram_tensor("ag_out", [NCORES, AGW], F32,
                            addr_space="Shared").ap()
    ag2_in = nc.dram_tensor("ag2_in", [1, 1], F32).ap()
    ag2_out = nc.dram_tensor("ag2_out", [NCORES, 1], F32,
                             addr_space="Shared").ap()

    with tile.TileContext(nc) as tc, ExitStack() as ctx:
        cp = ctx.enter_context(tc.tile_pool(name="const", bufs=1))

        # warm the gpsimd ext-isa library for partition_all_reduce (one-time
        # ~7.5us load) while input DMAs stream
        zz = cp.tile([128, 1], F32, tag="zz", name="zz")
        nc.vector.memset(zz[:, :], 0.0)
        zzo = cp.tile([128, 1], F32, tag="zzo", name="zzo")
        nc.gpsimd.partition_all_reduce(zzo[:, :], zz[:, :], channels=128,
                                       reduce_op=bass_isa.ReduceOp.max)

        # ---------- input DMAs: consts + x first, spread across queues ----
        ident16 = cp.tile([128, 128], F16, tag="ident16", name="ident16")
        nc.sync.dma_start(out=ident16[:, :], in_=ident16_d[:, :])
        ones_row = cp.tile([1, 128], F32, tag="ones_row", name="ones_row")
        nc.sync.dma_start(out=ones_row[:, :], in_=ones_row_d[:, :])
        ones_col = cp.tile([128, 1], F32, tag="ones_col", name="ones_col")
        nc.sync.dma_start(out=ones_col[:, :], in_=ones_col_d[:, :])
        asc = cp.tile([1, 2], F32, tag="asc", name="asc")
        nc.sync.dma_start(out=asc[:, 0:1], in_=a_model[0:1])
        nc.sync.dma_start(out=asc[:, 1:2], in_=a_hete[0:1])
        x16 = cp.tile([128, NT * INSM], F16, tag="x16", name="x16")
        x16v = x16[:, :].rearrange("p (i c) -> p i c", c=INSM)
        nc.sync.dma_start(
            out=x16v[:, 0:3, :],
            in_=x_sm[0:384, :].rearrange("(a p) c -> p a c", p=128))
        nc.scalar.dma_start(
            out=x16v[:, 3:6, :],
            in_=x_sm[384:768, :].rearrange("(a p) c -> p a c", p=128))

        # consts on gpsimd queue (must stay short: AG trigger lives here)
        ident = cp.tile([128, 128], F32, tag="ident", name="ident")
        nc.gpsimd.dma_start(out=ident[:, :], in_=ident_d[:, :])
        colmask = cp.tile([1, NSLOT], F32, tag="colmask", name="colmask")
        nc.gpsimd.dma_start(out=colmask[:, :], in_=colmask_d[:, :])

        # weights: batched DMAs on sync/scalar
        Wh0_t = cp.tile([128, 4 * HID], F16, tag="Wh0", name="Wh0")
        nc.sync.dma_start(
            out=Wh0_t[:, :].rearrange("p (a c) -> p a c", c=HID),
            in_=Wh0[:, :].rearrange("(a p) c -> p a c", p=128))
        Ws0_t = cp.tile([128, 4 * HID], F16, tag="Ws0", name="Ws0")
        nc.scalar.dma_start(
            out=Ws0_t[:, :].rearrange("p (a c) -> p a c", c=HID),
            in_=Ws0[:, :].rearrange("(a p) c -> p a c", p=128))
        Wh1_t = cp.tile([128, 2 * OUT], F16, tag="Wh1", name="Wh1")
        nc.sync.dma_start(
            out=Wh1_t[:, :].rearrange("p (a c) -> p a c", c=OUT),
            in_=Wh1[:, :].rearrange("(a p) c -> p a c", p=128))
        Ws1_t = cp.tile([128, 2 * OUT], F16, tag="Ws1", name="Ws1")
        nc.scalar.dma_start(
            out=Ws1_t[:, :].rearrange("p (a c) -> p a c", c=OUT),
            in_=Ws1[:, :].rearrange("(a p) c -> p a c", p=128))
        bh0_t = cp.tile([128, 2], F32, tag="bh0", name="bh0")
        nc.sync.dma_start(out=bh0_t[:, :].rearrange("p a -> p a"),
                          in_=bh0[:].rearrange("(a p) -> p a", p=128))
        bs0_t = cp.tile([128, 2], F32, tag="bs0", name="bs0")
        nc.scalar.dma_start(out=bs0_t[:, :].rearrange("p a -> p a"),
                            in_=bs0[:].rearrange("(a p) -> p a", p=128))
        bh1_r = cp.tile([1, OUT], F32, tag="bh1r", name="bh1r")
        nc.sync.dma_start(out=bh1_r[:, :], in_=bh1[:])
        bs1_r = cp.tile([1, OUT], F32, tag="bs1r", name="bs1r")
        nc.scalar.dma_start(out=bs1_r[:, :], in_=bs1[:])

        # ori branch loads (consumed late, in the AG2 window)
        x_o16 = cp.tile([128, NT * FEAT], F16, tag="xo16", name="xo16")
        nc.scalar.dma_start(
            out=x_o16[:, :].rearrange("p (a c) -> p a c", c=FEAT),
            in_=x_ori[:, :].rearrange("(a p) c -> p a c", p=128))
        Wl0_t = cp.tile([128, HID], F16, tag="Wl0", name="Wl0")
        nc.scalar.dma_start(out=Wl0_t[:, :], in_=Wl0[:, :])
        Wl1_t = cp.tile([128, 2 * OUT], F16, tag="Wl1", name="Wl1")
        nc.scalar.dma_start(
            out=Wl1_t[:, :].rearrange("p (a c) -> p a c", c=OUT),
            in_=Wl1[:, :].rearrange("(a p) c -> p a c", p=128))
        bl0_t = cp.tile([128, 2], F32, tag="bl0", name="bl0")
        nc.scalar.dma_start(out=bl0_t[:, :], in_=bl0[:].rearrange(
            "(a p) -> p a", p=128))
        bl1_r = cp.tile([1, OUT], F32, tag="bl1r", name="bl1r")
        nc.scalar.dma_start(out=bl1_r[:, :], in_=bl1[:])

        nbh0_t = cp.tile([128, 2], F32, tag="nbh0", name="nbh0")
        nc.vector.tensor_scalar(nbh0_t[:, :], bh0_t[:, :], -1.0, None, OP.mult)
        nbs0_t = cp.tile([128, 2], F32, tag="nbs0", name="nbs0")
        nc.vector.tensor_scalar(nbs0_t[:, :], bs0_t[:, :], -1.0, None, OP.mult)
        nbl0_t = cp.tile([128, 2], F32, tag="nbl0", name="nbl0")
        nc.vector.tensor_scalar(nbl0_t[:, :], bl0_t[:, :], -1.0, None, OP.mult)

        # ---------- broadcasts ----------
        aM = cp.tile([128, 1], F32, tag="aM", name="aM")
        aH = cp.tile([128, 1], F32, tag="aH", name="aH")
        naM = cp.tile([128, 1], F32, tag="naM", name="naM")
        naH = cp.tile([128, 1], F32, tag="naH", name="naH")
        bh1_b = cp.tile([128, OUT], F32, tag="bh1", name="bh1")
        bs1_b = cp.tile([128, OUT], F32, tag="bs1", name="bs1")
        bl1_b = cp.tile([128, OUT], F32, tag="bl1", name="bl1")
        with tc.tile_pool(name="ps_bc", bufs=2, space="PSUM") as ps0:
            for src, full, nfull in ((asc[:, 0:1], aM, naM),
                                     (asc[:, 1:2], aH, naH)):
                ps = ps0.tile([128, 1], F32, tag="ps_bc", name="ps_bc")
                nc.tensor.matmul(ps[:, :], ones_row[:, :], src,
                                 start=True, stop=True)
                nc.scalar.copy(full[:, :], ps[:, :])
                nc.vector.tensor_scalar(nfull[:, :], full[:, :], -1.0, None,
                                        OP.mult)
            for row, full in ((bh1_r, bh1_b), (bs1_r, bs1_b),
                              (bl1_r, bl1_b)):
                ps = ps0.tile([128, OUT], F32, tag="ps_b1", name="ps_b1")
                nc.tensor.matmul(ps[:, :], ones_row[:, :], row[:, :],
                                 start=True, stop=True)
                nc.scalar.copy(full[:, :], ps[:, :])

        # ---------- transpose x_sm (fp16, 1 cyc/row) ----------
        XT = [cp.tile([128, P], F16, tag=f"xt{f}", name=f"xt{f}")
              for f in range(4)]
        with tc.tile_pool(name="ps_tr", bufs=4, space="PSUM") as pst:
            for i in range(NT):
                for f in range(4):
                    pt = pst.tile([128, 128], F16, tag="tr", name="tr")
                    nc.tensor.transpose(
                        pt[:, :], x16v[:, i, 128 * f:128 * (f + 1)],
                        ident16[:, :])
                    if f % 2 == 0:
                        nc.scalar.copy(XT[f][:, 128 * i:128 * (i + 1)],
                                       pt[:, :])
                    else:
                        nc.vector.tensor_copy(XT[f][:, 128 * i:128 * (i + 1)],
                                              pt[:, :])

        def esl(i):
            return slice(OUT * i, OUT * (i + 1))

        # ---------- MLP helper (fp16 l1+l2 weights/inputs) ----------
        def mlp(XT_tiles, w0, b0t, nb0t, w1, b1_b, na_b, out_wide, pfx):
            with tc.tile_pool(name=pfx + "_h", bufs=1) as hp, \
                 tc.tile_pool(name=pfx + "_r", bufs=2) as rp, \
                 tc.tile_pool(name=pfx + "_ps1", bufs=2, space="PSUM") as ps1, \
                 tc.tile_pool(name=pfx + "_ps2", bufs=2, space="PSUM") as ps2:
                h_tiles = []
                for hs in range(2):
                    h = hp.tile([128, P], F16, tag=f"h{hs}",
                                name=f"{pfx}h{hs}")
                    for ns2 in range(2):
                        sl = slice(384 * ns2, 384 * (ns2 + 1))
                        pp = ps1.tile([128, 384], F32, tag="l1", name="l1")
                        nf = len(XT_tiles)
                        for fc in range(nf):
                            nc.tensor.matmul(pp[:, :], w0(fc, hs),
                                             XT_tiles[fc][:, sl],
                                             start=(fc == 0),
                                             stop=(fc == nf - 1))
                        r1 = rp.tile([128, 384], F32, tag="r1", name="r1")
                        nc.scalar.activation(r1[:, :], pp[:, :], AF.Relu,
                                             bias=b0t[:, hs:hs + 1],
                                             scale=1.0)
                        r2 = rp.tile([128, 384], F32, tag="r2", name="r2")
                        nc.scalar.activation(r2[:, :], pp[:, :], AF.Relu,
                                             bias=nb0t[:, hs:hs + 1],
                                             scale=-1.0)
                        nc.vector.scalar_tensor_tensor(
                            h[:, sl], r2[:, :], na_b[:, :], r1[:, :],
                            op0=OP.mult, op1=OP.add)
                    h_tiles.append(h)
                for i in range(NT):
                    pp = ps2.tile([128, OUT], F32, tag="l2", name="l2")
                    for hs in range(2):
                        nc.tensor.matmul(pp[:, :],
                                         h_tiles[hs][:, 128 * i:128 * (i + 1)],
                                         w1(hs), start=(hs == 0),
                                         stop=(hs == 1))
                    nc.vector.scalar_tensor_tensor(
                        out_wide[:, esl(i)], pp[:, :], 1.0, b1_b[:, :],
                        op0=OP.mult, op1=OP.add)

        def w0h(fc, hs):
            return Wh0_t[:, HID * fc + 128 * hs:HID * fc + 128 * (hs + 1)]

        def w0s(fc, hs):
            return Ws0_t[:, HID * fc + 128 * hs:HID * fc + 128 * (hs + 1)]

        def w1h(hs):
            return Wh1_t[:, OUT * hs:OUT * (hs + 1)]

        def w1s(hs):
            return Ws1_t[:, OUT * hs:OUT * (hs + 1)]

        # persistent phase-1 outputs
        emb_loc = cp.tile([128, NT * OUT], F32, tag="emb_loc", name="emb_loc")
        emb16_loc = cp.tile([128, NT * OUT], F16, tag="e16l", name="e16l")
        tT16 = cp.tile([128, P], F16, tag="tT16", name="tT16")
        tw = cp.tile([128, NT * OUT], F32, tag="tw", name="tw")
        maxs = cp.tile([128, NSLOT], F32, tag="maxs", name="maxs")
        nc.vector.memset(tT16[64:128, :], 0.0)

        # hete MLP first (feeds the collective)
        mlp(XT, w0h, bh0_t, nbh0_t, w1h, bh1_b, naH, emb_loc, "hete")
        nc.vector.tensor_copy(emb16_loc[:, :], emb_loc[:, :])

        # AG-emb goes out as soon as the hete MLP is done
        nc.sync.dma_start(out=age_in[0:1, 0:AGE],
                          in_=emb16_loc[:, :].bitcast(F32))
        nc.gpsimd.collective_compute(
            "AllGather", OP.bypass,
            ins=[age_in[:, :]],
            outs=[age_out[:, :]],
            replica_groups=[list(range(NCORES))])

        # ---------- softmax -> t = e/||e|| (fp16), wr, u ----------
        wr_sb = cp.tile([64, 64], F32, tag="wr_sb", name="wr_sb")
        u_sb = cp.tile([1, 64], F32, tag="u_sb", name="u_sb")
        with tc.tile_pool(name="smax", bufs=1) as sp, \
             tc.tile_pool(name="ps_wr", bufs=1, space="PSUM") as pswr, \
             tc.tile_pool(name="ps_tr2", bufs=2, space="PSUM") as pst2:
            rmx = sp.tile([128, NT], F32, tag="rmx", name="rmx")
            nc.vector.tensor_reduce(
                rmx[:, :],
                emb_loc[:, :].rearrange("p (g o) -> p g o", o=OUT),
                axis=AX.X, op=OP.max, negate=True)
            ex_w = sp.tile([128, NT * OUT], F32, tag="ex_w", name="ex_w")
            for i in range(NT):
                nc.scalar.activation(ex_w[:, esl(i)], emb_loc[:, esl(i)],
                                     AF.Exp, bias=rmx[:, i:i + 1], scale=1.0)
            sq_w = sp.tile([128, NT * OUT], F32, tag="sq_w", name="sq_w")
            nc.vector.tensor_mul(sq_w[:, :], ex_w[:, :], ex_w[:, :])
            dsum = sp.tile([128, NT], F32, tag="dsum", name="dsum")
            nc.vector.tensor_reduce(
                dsum[:, :],
                sq_w[:, :].rearrange("p (g o) -> p g o", o=OUT),
                axis=AX.X, op=OP.add)
            rd = sp.tile([128, NT], F32, tag="rd", name="rd")
            nc.vector.reciprocal(rd[:, :], dsum[:, :])
            isd = sp.tile([128, NT], F32, tag="isd", name="isd")
            nc.scalar.activation(isd[:, :], rd[:, :], AF.Sqrt)
            t16 = sp.tile([128, NT * OUT], F16, tag="t16", name="t16")
            for i in range(NT):
                nc.vector.tensor_scalar(t16[:, esl(i)], ex_w[:, esl(i)],
                                        isd[:, i:i + 1], None, OP.mult)
            ps_wr = pswr.tile([64, 64], F32, tag="wr", name="pswr")
            for i in range(NT):
                nc.tensor.matmul(ps_wr[:, :], t16[:, esl(i)],
                                 emb16_loc[:, esl(i)],
                                 start=(i == 0), stop=(i == NT - 1))
                pt = pst2.tile([64, 128], F16, tag="ttr", name="ttr")
                nc.tensor.transpose(pt[:, :], t16[:, esl(i)], ident16[:, :])
                nc.scalar.copy(tT16[0:64, 128 * i:128 * (i + 1)], pt[:, :])
            nc.scalar.copy(wr_sb[:, :], ps_wr[:, :])
            # u = colsum(t): rowsum of tT16, transposed to a row
            uT = sp.tile([64, 1], F32, tag="uT", name="uT")
            nc.vector.tensor_reduce(uT[:, :], tT16[0:64, :], axis=AX.X,
                                    op=OP.add)
            pu = pst2.tile([1, 64], F32, tag="put", name="put")
            nc.tensor.transpose(pu[:, :], uT[:, :], ident[0:64, 0:64])
            nc.scalar.copy(u_sb[:, :], pu[:, :])

        # ---------- pack + AG-t ----------
        nc.sync.dma_start(out=ag_in[0:1, 0:OFF_W],
                          in_=tT16[0:64, :].bitcast(F32))
        nc.sync.dma_start(out=ag_in[0:1, OFF_W:OFF_U], in_=wr_sb[:, :])
        nc.scalar.dma_start(out=ag_in[0:1, OFF_U:AGW], in_=u_sb[:, :])
        nc.gpsimd.collective_compute(
            "AllGather", OP.bypass,
            ins=[ag_in[:, :]],
            outs=[ag_out[:, :]],
            replica_groups=[list(range(NCORES))])

        # ---------- AG bubble: local-block max tiles + smooth MLP ----------
        negbigI = cp.tile([128, 128], F32, tag="negbigI", name="negbigI")
        nc.vector.tensor_scalar(negbigI[:, :], ident[:, :], -BIG, None,
                                OP.mult)
        with tc.tile_pool(name="ps_rex", bufs=2, space="PSUM") as psre:
            for s in range(NT):
                for h in range(2):
                    sl = slice(384 * h, 384 * (h + 1))
                    pp = psre.tile([128, 384], F32, tag="rex", name="rex")
                    nc.tensor.matmul(pp[:, :],
                                     tT16[:, 128 * s:128 * (s + 1)],
                                     tT16[:, sl], start=True, stop=True)
                    if (s // 3) == h:
                        off = 128 * s - 384 * h
                        nc.vector.scalar_tensor_tensor(
                            pp[:, off:off + 128], ident[:, :], -BIG,
                            pp[:, off:off + 128], op0=OP.mult, op1=OP.add)
                    slot = 7 + 2 * s + h
                    nc.vector.tensor_reduce(maxs[:, slot:slot + 1], pp[:, :],
                                            axis=AX.X, op=OP.max)
        with tc.tile_pool(name="mlpout", bufs=1) as mo:
            sm_out = mo.tile([128, NT * OUT], F32, tag="smo", name="smo")
            mlp(XT, w0s, bs0_t, nbs0_t, w1s, bs1_b, naM, sm_out, "smooth")
            nc.sync.dma_start(
                out=out_smooth[:, :].rearrange("(a p) c -> p a c", p=128),
                in_=sm_out[:, :].rearrange("p (a c) -> p a c", c=OUT))

        # ori MLP also fills the AG window
        XTo = cp.tile([128, P], F16, tag="xto", name="xto")
        with tc.tile_pool(name="ps_or", bufs=2, space="PSUM") as pso:
            for i in range(NT):
                pt = pso.tile([128, 128], F16, tag="otr", name="otr")
                nc.tensor.transpose(
                    pt[:, :],
                    x_o16[:, :].rearrange("p (a c) -> p a c",
                                          c=FEAT)[:, i, :],
                    ident16[:, :])
                nc.scalar.copy(XTo[:, 128 * i:128 * (i + 1)], pt[:, :])

        def w0l(fc, hs):
            return Wl0_t[:, 128 * hs:128 * (hs + 1)]

        def w1l(hs):
            return Wl1_t[:, OUT * hs:OUT * (hs + 1)]

        with tc.tile_pool(name="mlpout2", bufs=1) as mo2:
            or_out = mo2.tile([128, NT * OUT], F32, tag="oro", name="oro")
            mlp([XTo], w0l, bl0_t, nbl0_t, w1l, bl1_b, naM, or_out, "ori")
            nc.sync.dma_start(
                out=out_ori[:, :].rearrange("(a p) c -> p a c", p=128),
                in_=or_out[:, :].rearrange("p (a c) -> p a c", c=OUT))

        # ---------- unpack AG-emb + AG-t ----------
        tf16 = [cp.tile([128, P], F16, tag=f"tf{k}", name=f"tf{k}")
                for k in range(NCORES)]
        emb16 = [cp.tile([128, NT * OUT], F16, tag=f"e16_{k}",
                         name=f"e16_{k}") for k in range(NCORES)]
        w16 = cp.tile([64, 64], F16, tag="w16", name="w16")
        mb = cp.tile([128, 1], F32, tag="mb", name="mb")
        nmb = cp.tile([128, 1], F32, tag="nmb", name="nmb")
        # u gather FIRST: it unlocks the m chain / fused start.
        # uall8 [8,64] across partitions; U via two tiny PE matmuls.
        with tc.tile_pool(name="unpack", bufs=1) as up, \
             tc.tile_pool(name="ps_m", bufs=2, space="PSUM") as psm:
            uall8 = up.tile([8, 64], F32, tag="uall8", name="uall8")
            nc.sync.dma_start(out=uall8[:, :], in_=ag_out[:, OFF_U:AGW])
            pU = psm.tile([64, 1], F32, tag="pU", name="pU")
            nc.tensor.matmul(pU[:, :], uall8[:, :], ones_col[0:8, :],
                             start=True, stop=True)
            UT = up.tile([64, 1], F32, tag="UT", name="UT")
            nc.scalar.copy(UT[:, :], pU[:, :])
            puu = psm.tile([1, 1], F32, tag="puu", name="puu")
            nc.tensor.matmul(puu[:, :], UT[:, :], UT[:, :],
                             start=True, stop=True)
            m01 = up.tile([1, 1], F32, tag="m01", name="m01")
            nc.vector.tensor_scalar(m01[:, :], puu[:, :], -float(N), INV_N2,
                                    OP.add, OP.mult)
            pb = psm.tile([128, 1], F32, tag="mbc", name="mbc")
            nc.tensor.matmul(pb[:, :], ones_row[:, :], m01[:, :],
                             start=True, stop=True)
            nc.scalar.copy(mb[:, :], pb[:, :])
            nc.vector.tensor_scalar(nmb[:, :], mb[:, :], -1.0, None, OP.mult)
            # big per-core unpacks (k-ascending: the fused loop chases them)
            for k in range(NCORES):
                nc.vector.memset(tf16[k][64:128, :], 0.0)
                eng = nc.sync if k % 2 == 0 else nc.scalar
                eng2 = nc.scalar if k % 2 == 0 else nc.sync
                eng.dma_start(out=tf16[k][0:64, :].bitcast(F32),
                              in_=ag_out[k:k + 1, 0:OFF_W])
                eng2.dma_start(out=emb16[k][:, :].bitcast(F32),
                               in_=age_out[k:k + 1, 0:AGE])
            # wr gather (needed only in the epilogue)
            wrall = up.tile([64, 8 * 64], F32, tag="wrall", name="wrall")
            nc.scalar.dma_start(
                out=wrall[:, :].rearrange("p (k c) -> p k c", k=NCORES),
                in_=ag_out[:, OFF_W:OFF_U].rearrange("k (p c) -> p k c",
                                                     p=64))
            w_sb = up.tile([64, 64], F32, tag="w_sb", name="w_sb")
            nc.vector.tensor_reduce(
                w_sb[:, :],
                wrall[:, :].rearrange("p (k c) -> p c k", k=NCORES),
                axis=AX.X, op=OP.add)
            nc.vector.tensor_copy(w16[:, :], w_sb[:, :])

        # epilogue scalars that only need m / aH
        nimb = cp.tile([128, 1], F32, tag="nimb", name="nimb")  # -1/m
        imb = cp.tile([128, 1], F32, tag="imb", name="imb")     # 1/m
        omaH = cp.tile([128, 1], F32, tag="omaH", name="omaH")  # 1-aH
        n1m = cp.tile([128, 1], F32, tag="n1m", name="n1m")     # m-1
        nc.vector.reciprocal(imb[:, :], mb[:, :])
        nc.vector.tensor_scalar(nimb[:, :], imb[:, :], -1.0, None, OP.mult)
        nc.vector.tensor_scalar(omaH[:, :], aH[:, :], -1.0, 1.0, OP.mult,
                                OP.add)
        nc.vector.tensor_scalar(n1m[:, :], mb[:, :], 1.0, -1.0, OP.mult,
                                OP.add)

        # ---------- fused relation + propagation pass ----------
        qeT = cp.tile([64, P], F32, tag="qeT", name="qeT")
        seT = cp.tile([64, P], F32, tag="seT", name="seT")
        with tc.tile_pool(name="ps_re", bufs=3, space="PSUM") as psre, \
             tc.tile_pool(name="ps_acc", bufs=1, space="PSUM") as pacc, \
             tc.tile_pool(name="qc", bufs=3) as qcp, \
             tc.tile_pool(name="sg", bufs=3) as sgp, \
             tc.tile_pool(name="rmp", bufs=2) as rmp:
            qe_ps = [pacc.tile([64, 384], F32, tag=f"qe{h}", name=f"qe{h}")
                     for h in range(2)]
            se_ps = [pacc.tile([64, 384], F32, tag=f"se{h}", name=f"se{h}")
                     for h in range(2)]
            for k in range(NCORES):
                rm = rmp.tile([128, P], F16, tag="rm", name="rm")
                for sub in range(NT):
                    j = NT * k + sub
                    q2 = qcp.tile([128, P], F16, tag="q2", name="q2")
                    s2 = sgp.tile([128, P], F16, tag="s2", name="s2")
                    pps = []
                    for h in range(2):
                        pp = psre.tile([128, 384], F32, tag="rem",
                                       name="rem")
                        nc.tensor.matmul(
                            pp[:, :],
                            tf16[k][:, 128 * sub:128 * (sub + 1)],
                            tT16[:, 384 * h:384 * (h + 1)],
                            start=True, stop=True)
                        pps.append(pp)
                    for h in range(2):
                        qsl = slice(384 * h, 384 * (h + 1))
                        nc.scalar.activation(q2[:, qsl], pps[h][:, :],
                                             AF.Relu, bias=nmb[:, :],
                                             scale=1.0)
                        # per-half sigma so se_h never waits on the
                        # other q-half (keeps the 4 accums contiguous)
                        nc.vector.tensor_scalar(s2[:, qsl], q2[:, qsl],
                                                0.0, None, OP.is_gt)
                    if k < 7:
                        if sub == 0:
                            nc.vector.tensor_scalar(rm[:, :], q2[:, :],
                                                    0.0, None, OP.max)
                        else:
                            nc.vector.tensor_max(rm[:, :], rm[:, :],
                                                 q2[:, :])
                    for h in range(2):
                        qsl = slice(384 * h, 384 * (h + 1))
                        nc.tensor.matmul(qe_ps[h][:, :],
                                         emb16[k][:, esl(sub)], q2[:, qsl],
                                         start=(j == 0), stop=(j == NJ - 1),
                                         skip_group_check=True)
                        nc.tensor.matmul(se_ps[h][:, :],
                                         emb16[k][:, esl(sub)], s2[:, qsl],
                                         start=(j == 0), stop=(j == NJ - 1),
                                         skip_group_check=True)
                if k < 7:
                    nc.vector.tensor_reduce(maxs[:, k:k + 1], rm[:, :],
                                            axis=AX.X, op=OP.max)
                if k == 6:
                    # k=7 slot is covered by peers (re is symmetric):
                    # stats + AG2 launch overlap the last fused chunk
                    nc.vector.tensor_scalar(maxs[:, 7:NSLOT],
                                            maxs[:, 7:NSLOT], mb[:, :],
                                            0.0, OP.subtract, OP.max)
                    mall = cp.tile([128, NSLOT], F32, tag="mall",
                                   name="mall")
                    nc.gpsimd.partition_all_reduce(
                        mall[:, :], maxs[:, :], channels=128,
                        reduce_op=bass_isa.ReduceOp.max)
                    mrow = cp.tile([1, NSLOT], F32, tag="mrow", name="mrow")
                    nc.vector.tensor_mul(mrow[:, :], mall[0:1, :],
                                         colmask[:, :])
                    mx01 = cp.tile([1, 1], F32, tag="mx01", name="mx01")
                    nc.vector.tensor_reduce(mx01[:, :], mrow[:, :],
                                            axis=AX.X, op=OP.max)
                    nc.sync.dma_start(out=ag2_in[:, :], in_=mx01[:, :])
                    nc.gpsimd.collective_compute(
                        "AllGather", OP.bypass,
                        ins=[ag2_in[:, :]],
                        outs=[ag2_out[:, :]],
                        replica_groups=[list(range(NCORES))])
            for h in range(2):
                sl = slice(384 * h, 384 * (h + 1))
                nc.scalar.copy(qeT[:, sl], qe_ps[h][:, :])
                nc.scalar.copy(seT[:, sl], se_ps[h][:, :])

        # ---------- epilogue prep (independent of qmax) ----------
        with tc.tile_pool(name="epi", bufs=1) as ep, \
             tc.tile_pool(name="ps_epi", bufs=2, space="PSUM") as pse:
            qe_nm = ep.tile([128, NT * OUT], F32, tag="qe_nm", name="qe_nm")
            se_nm = ep.tile([128, NT * OUT], F32, tag="se_nm", name="se_nm")
            for i in range(NT):
                pp = pse.tile([128, OUT], F32, tag="twp", name="twp")
                nc.tensor.matmul(pp[:, :], tT16[0:64, 128 * i:128 * (i + 1)],
                                 w16[:, :], start=True, stop=True)
                nc.scalar.copy(tw[:, esl(i)], pp[:, :])
            for i in range(NT):
                csl = slice(128 * i, 128 * (i + 1))
                pq = pse.tile([128, 64], F32, tag="tq", name="tq")
                nc.tensor.transpose(pq[:, :], qeT[:, csl], ident[0:64, 0:64])
                nc.scalar.copy(qe_nm[:, esl(i)], pq[:, :])
                pc = pse.tile([128, 64], F32, tag="tc", name="tc")
                nc.tensor.transpose(pc[:, :], seT[:, csl], ident[0:64, 0:64])
                nc.scalar.copy(se_nm[:, esl(i)], pc[:, :])

            z1 = ep.tile([128, NT * OUT], F32, tag="z1", name="z1")
            nc.vector.scalar_tensor_tensor(z1[:, :], emb_loc[:, :], n1m[:, :],
                                           qe_nm[:, :], op0=OP.mult,
                                           op1=OP.add)
            z2 = ep.tile([128, NT * OUT], F32, tag="z2", name="z2")
            nc.vector.tensor_scalar(z2[:, :], qe_nm[:, :], imb[:, :], None,
                                    OP.mult)
            nc.vector.tensor_add(z2[:, :], z2[:, :], se_nm[:, :])
            nc.vector.scalar_tensor_tensor(z2[:, :], tw[:, :], nimb[:, :],
                                           z2[:, :], op0=OP.mult, op1=OP.add)
            nc.vector.tensor_add(z2[:, :], z2[:, :], emb_loc[:, :])
            t2 = ep.tile([128, NT * OUT], F32, tag="t2", name="t2")
            nc.vector.tensor_scalar(t2[:, :], emb_loc[:, :], omaH[:, :],
                                    None, OP.mult)
            cpos = ep.tile([128, NT * OUT], F32, tag="cpos", name="cpos")
            nc.vector.scalar_tensor_tensor(cpos[:, :], z2[:, :], aH[:, :],
                                           t2[:, :], op0=OP.mult, op1=OP.add)
            cneg = ep.tile([128, NT * OUT], F32, tag="cneg", name="cneg")
            nc.vector.tensor_sub(cneg[:, :], t2[:, :], z2[:, :])
            ehalf = ep.tile([128, NT * OUT], F32, tag="ehalf", name="ehalf")
            nc.vector.tensor_scalar(ehalf[:, :], emb_loc[:, :], 0.5, None,
                                    OP.mult)

            # ---------- qmax -> ip ----------
            ipb = cp.tile([128, 1], F32, tag="ipb", name="ipb")
            naip = cp.tile([128, 1], F32, tag="naip", name="naip")
            with tc.tile_pool(name="glob", bufs=1) as gp, \
                 tc.tile_pool(name="ps_gl", bufs=1, space="PSUM") as psg:
                m8 = gp.tile([1, 8], F32, tag="m8", name="m8")
                nc.sync.dma_start(out=m8[:, :], in_=ag2_out[:, 0:1])
                mxs = gp.tile([1, 1], F32, tag="mxs", name="mxs")
                nc.vector.tensor_reduce(mxs[:, :], m8[:, :], axis=AX.X,
                                        op=OP.max)
                pb = psg.tile([128, 1], F32, tag="bc", name="bc")
                nc.tensor.matmul(pb[:, :], ones_row[:, :], mxs[:, :],
                                 start=True, stop=True)
                pd = gp.tile([128, 1], F32, tag="pd", name="pd")
                nc.scalar.copy(pd[:, :], pb[:, :])
                nc.vector.reciprocal(ipb[:, :], pd[:, :])
                nc.vector.tensor_mul(naip[:, :], ipb[:, :], naH[:, :])

            # both branches side by side in one wide tile
            pw2 = ep.tile([128, 2 * NT * OUT], F32, tag="pw2", name="pw2")
            nc.vector.scalar_tensor_tensor(pw2[:, 0:384], z1[:, :],
                                           ipb[:, :], cpos[:, :],
                                           op0=OP.mult, op1=OP.add)
            nc.vector.scalar_tensor_tensor(pw2[:, 384:768], z1[:, :],
                                           naip[:, :], cneg[:, :],
                                           op0=OP.mult, op1=OP.add)
            rmx2 = ep.tile([128, 2 * NT], F32, tag="rmx2", name="rmx2")
            nc.vector.tensor_reduce(
                rmx2[:, :],
                pw2[:, :].rearrange("p (g o) -> p g o", o=OUT),
                axis=AX.X, op=OP.max, negate=True)
            ex2 = ep.tile([128, 2 * NT * OUT], F32, tag="ex2", name="ex2")
            for i in range(2 * NT):
                nc.scalar.activation(ex2[:, esl(i)], pw2[:, esl(i)],
                                     AF.Exp, bias=rmx2[:, i:i + 1],
                                     scale=1.0)
            ssum2 = ep.tile([128, 2 * NT], F32, tag="ssum2", name="ssum2")
            nc.vector.tensor_reduce(
                ssum2[:, :],
                ex2[:, :].rearrange("p (g o) -> p g o", o=OUT),
                axis=AX.X, op=OP.add)
            rs2 = ep.tile([128, 2 * NT], F32, tag="rs2", name="rs2")
            nc.vector.reciprocal(rs2[:, :], ssum2[:, :])
            pp_w = ep.tile([128, NT * OUT], F32, tag="pp_w", name="pp_w")
            pn_w = ep.tile([128, NT * OUT], F32, tag="pn_w", name="pn_w")
            for i in range(NT):
                nc.vector.tensor_scalar(pp_w[:, esl(i)], ex2[:, esl(i)],
                                        rs2[:, i:i + 1], None, OP.mult)
                nc.vector.tensor_scalar(pn_w[:, esl(i)],
                                        ex2[:, 384 + OUT * i:384 + OUT *
                                            (i + 1)],
                                        rs2[:, NT + i:NT + i + 1], None,
                                        OP.mult)
            dd = ep.tile([128, NT * OUT], F32, tag="dd", name="dd")
            nc.vector.tensor_sub(dd[:, :], pp_w[:, :], pn_w[:, :])
            msg = ep.tile([128, NT * OUT], F32, tag="msg", name="msg")
            nc.vector.scalar_tensor_tensor(msg[:, :], dd[:, :], 0.5,
                                           ehalf[:, :], op0=OP.mult,
                                           op1=OP.add)
            nc.sync.dma_start(
                out=out_msg[:, :].rearrange("(a p) c -> p a c", p=128),
                in_=msg[:, :].rearrange("p (a c) -> p a c", c=OUT))

    nc.compile()
    return nc


def _get_nc():
    if "nc" not in _CACHE:
        _CACHE["nc"] = _build()
    return _CACHE["nc"]


def _make_in_maps(inputs):
    f = np.float32
    h = np.float16
    sm = np.ascontiguousarray(inputs["smoothed_feature"], dtype=h)
    ori = np.ascontiguousarray(inputs["ori_feature"], dtype=h)
    shared = {
        "Wh016": np.ascontiguousarray(inputs["W_hete0"], dtype=h),
        "Ws016": np.ascontiguousarray(inputs["W_smooth0"], dtype=h),
        "Wl016": np.ascontiguousarray(inputs["W_local0"], dtype=h),
        "Wh116": np.ascontiguousarray(inputs["W_hete1"], dtype=h),
        "Ws116": np.ascontiguousarray(inputs["W_smooth1"], dtype=h),
        "Wl116": np.ascontiguousarray(inputs["W_local1"], dtype=h),
        "b_hete0": np.ascontiguousarray(inputs["b_hete0"], dtype=f),
        "b_hete1": np.ascontiguousarray(inputs["b_hete1"], dtype=f),
        "b_smooth0": np.ascontiguousarray(inputs["b_smooth0"], dtype=f),
        "b_smooth1": np.ascontiguousarray(inputs["b_smooth1"], dtype=f),
        "b_local0": np.ascontiguousarray(inputs["b_local0"], dtype=f),
        "b_local1": np.ascontiguousarray(inputs["b_local1"], dtype=f),
        "prelu_model": np.ascontiguousarray(inputs["prelu_model"], dtype=f),
        "prelu_hete": np.ascontiguousarray(inputs["prelu_hete"], dtype=f),
        "ident": np.eye(128, dtype=f),
        "ident16": np.eye(128, dtype=h),
        "ones_row": np.ones((1, 128), dtype=f),
        "ones_col": np.ones((128, 1), dtype=f),
    }
    in_maps = []
    for r in range(NCORES):
        cm = np.ones((1, NSLOT), dtype=f)
        if r < 7:
            cm[0, r] = 0.0  # drop the raw local-block slot
        # rank 7's own block is k=7, which has no raw slot at all
        m = dict(shared)
        m["x_sm16"] = np.ascontiguousarray(sm[P * r:P * (r + 1)])
        m["x_ori16"] = np.ascontiguousarray(ori[P * r:P * (r + 1)])
        m["colmask"] = cm
        in_maps.append(m)
    return in_maps


def _ensure_ntff_hook():
    """The agent image's antenv lacks axon_hooks; shim it so
    run_bass_kernel_spmd(trace=True) can capture NTFF profiles."""
    if "antenv.axon_hooks" in sys.modules:
        return
    import types
    import antenv
    mod = types.ModuleType("antenv.axon_hooks")
    state = {"hook": None}
    mod.set_axon_ntff_profile_hook = lambda h: state.__setitem__("hook", h)
    mod.get_axon_ntff_profile_hook = lambda: state["hook"]
    sys.modules["antenv.axon_hooks"] = mod
    antenv.axon_hooks = mod
    try:
        from trn_agent_boot.trn_boot import _ntff_profile_via_ctypes
        mod.set_axon_ntff_profile_hook(
            _ntff_profile_via_ctypes("/opt/axon/libaxon_pjrt.so"))
    except Exception as e:
        print(f"ntff hook install failed: {e}", file=sys.stderr)


def run(inputs, trace=False):
    if trace:
        _ensure_ntff_hook()
    nc = _get_nc()
    in_maps = _make_in_maps(inputs)
    res = run_bass_kernel_spmd(nc, in_maps, list(range(NCORES)), trace=trace)
    outs = res.results
    o1 = np.concatenate([outs[r]["out_ori"] for r in range(NCORES)], axis=0)
    o2 = np.concatenate([outs[r]["out_smooth"] for r in range(NCORES)], axis=0)
    o3 = np.concatenate([outs[r]["out_msg"] for r in range(NCORES)], axis=0)
    return (o1.astype(np.float32), o2.astype(np.float32),
            o3.astype(np.float32)), res


def kernel(**inputs):
    (o1, o2, o3), _ = run(inputs, trace=False)
    return (o1, o2, o3)
